# revision 5
# baseline (speedup 1.0000x reference)
"""Trainium2 Bass kernel for batch-8 multi-head attention.

Strategy: pure data parallelism — one batch element per NeuronCore (B=8,
8 cores), zero collectives.  All inputs are pre-arranged on the host so the
device kernel only ever runs dense matmuls in its preferred layouts.

  per-core DRAM inputs (bf16 unless noted):
    xT        [128, 6, 1024]  x[b].T k-chunked       (feature-major activations)
    wqk0a     [128, 256]      pair-0 (m=0,6) cols of [W_q*SCALE | W_k], k=0
    wqk0b     [128, 1280]     pair-0 cols, k=1..5
    wqkA      [128, 3072]     pairs 1-2 cols, k-major
    wqkB      [128, 4608]     pairs 3-5 cols, k-major
    wv        [128, 4608]     W_v k-major
    wp        [128, 4608]     W_proj k-major
    b_all     [128, 18] f32   per-partition bias chunks (12 qk + 6 proj)
    b_v       [128, 12, 64] f32  V bias broadcast along partitions
  output:
    outT      [768, 1024] bf16  (attention output)^T — host transposes back

Device pipeline per core (program order interleaves phases so ScalarE's exp
stream starts early and the PE never starves — PE occupancy is the
bottleneck: ~165us of matmul on a ~143.5us column-count floor):
  QK^T = w_qk^T @ xT                 -> 12 tiles [128, 1024], 2 heads/tile
  V    = xT^T @ w_v + b_v            -> 8 tiles [128, 12, 65], ones col fused
  per head h:
    S^T[m]   = K_h @ Q_h^T           (K=64 contraction)
    expS     = exp(S^T)  on ScalarE  (no max subtraction: |logits| < ~8)
    [O^T|s]  = [V_m|1]^T @ expS      (PSUM accumulate over m; row 64 = sums)
    oT       = O^T * (1/s)           (recip_approx + partition-broadcast + mul)
  outT = w_proj^T @ oT + b_proj      (partials m0-3 early, m4 after heads 8/9,
                                      m5-only tail after the last normalize)

DMA plan: input dma_starts are spread across the scalar/sync/vector/gpsimd
sequencers (one DIRECT2D is ~610ns of sequencer time — serializing 26 of
them on SP delayed PE saturation to ~25us).  Output chunks round-robin over
all four sequencers as they complete.

PSUM: s-tiles 2-bank x2 + o-accumulators 1-bank x2 + filler chunks 1-bank x2
= 8 banks, shared across all phases with no pool barrier.

Attempts that CORRUPT on hardware (sim-clean, do not retry): genuinely
overlapping 64-row-tiled S matmuls with neighboring 128-mode matmuls
(tile_position + early PSUM release), [65,512] DVE copies from PSUM, and
reciprocal/mul reading o_ps PSUM directly.  Finer [128,512] exps are
correct but slower (+287ns fixed cost per activation instruction).
"""

import os
import sys

os.environ.setdefault("BASS_PERFETTO_PROFILE_ALL_CORES", "1")
if "/opt/trn_rl_repo" not in sys.path:
    sys.path.insert(0, "/opt/trn_rl_repo")

import numpy as np
import ml_dtypes

B, N, C, H = 8, 1024, 768, 12
D = C // H                # 64 head dim
SCALE = D ** -0.5
NCORES = 8
KT = C // 128             # 6 contraction tiles over C
MT = N // 128             # 8 token blocks
NJ = N // 512             # 2 query chunks of 512
BF16 = ml_dtypes.bfloat16

_CACHE = {}


def build_nc():
    """Build + compile the per-core Bass graph (identical on all 8 cores)."""
    import concourse.tile as tile
    from concourse import bacc, mybir

    f32 = mybir.dt.float32
    bf16 = mybir.dt.bfloat16
    Exp = mybir.ActivationFunctionType.Exp
    CopyF = mybir.ActivationFunctionType.Copy

    nc = bacc.Bacc("TRN2", target_bir_lowering=False, debug=False,
                   num_devices=NCORES)

    xT_e = nc.dram_tensor("xT", [128, KT, N], bf16, kind="ExternalInput").ap()
    wqk0a_e = nc.dram_tensor("wqk0a", [128, 256], bf16, kind="ExternalInput").ap()
    wqk0b_e = nc.dram_tensor("wqk0b", [128, 1280], bf16, kind="ExternalInput").ap()
    wqkA_e = nc.dram_tensor("wqkA", [128, KT * 512], bf16, kind="ExternalInput").ap()
    wqkB_e = nc.dram_tensor("wqkB", [128, KT * 768], bf16, kind="ExternalInput").ap()
    wv_e = nc.dram_tensor("w_v", [128, KT * C], bf16, kind="ExternalInput").ap()
    wp_e = nc.dram_tensor("w_proj", [128, KT * C], bf16, kind="ExternalInput").ap()
    ball_e = nc.dram_tensor("b_all", [128, 18], f32, kind="ExternalInput").ap()
    bv_e = nc.dram_tensor("b_v", [128, H, D], f32, kind="ExternalInput").ap()
    out_e = nc.dram_tensor("outT", [C, N], bf16, kind="ExternalOutput").ap()

    with tile.TileContext(nc) as tc:
        from contextlib import ExitStack

        with ExitStack() as es:
            persist = es.enter_context(tc.tile_pool(name="persist", bufs=1))
            s_pool = es.enter_context(tc.tile_pool(name="spsum", bufs=2, space="PSUM"))
            o_pool = es.enter_context(tc.tile_pool(name="opsum", bufs=2, space="PSUM"))
            f_pool = es.enter_context(tc.tile_pool(name="fpsum", bufs=2, space="PSUM"))
            e_pool = es.enter_context(tc.tile_pool(name="expS", bufs=14))
            r_pool = es.enter_context(tc.tile_pool(name="recip", bufs=2))
            st_pool = es.enter_context(tc.tile_pool(name="stage", bufs=3))
            rb_pool = es.enter_context(tc.tile_pool(name="recipb", bufs=2))
            out_pool = es.enter_context(tc.tile_pool(name="outc", bufs=3))

            # ---- persistent SBUF tiles ----------------------------------
            xT = [persist.tile([128, N], bf16, name=f"xT{k}", tag=f"xT{k}")
                  for k in range(KT)]
            wqk0a = persist.tile([128, 256], bf16, name="wqk0a", tag="wqk0a")
            wqk0b = persist.tile([128, 1280], bf16, name="wqk0b", tag="wqk0b")
            wqkA = persist.tile([128, KT * 512], bf16, name="wqkA", tag="wqkA")
            wqkB = persist.tile([128, KT * 768], bf16, name="wqkB", tag="wqkB")
            wv = persist.tile([128, KT * C], bf16, name="wv", tag="wv")
            wp = persist.tile([128, KT * C], bf16, name="wp", tag="wp")
            ball = persist.tile([128, 18], f32, name="ball", tag="ball")
            bv = persist.tile([128, H, D], f32, name="bv", tag="bv")
            qkT = [persist.tile([128, N], bf16, name=f"qkT{m}", tag=f"qkT{m}")
                   for m in range(12)]
            v_sb = [persist.tile([128, H, D + 1], bf16, name=f"v{t}", tag=f"v{t}")
                    for t in range(MT)]
            oT = [persist.tile([128, N], bf16, name=f"oT{m}", tag=f"oT{m}")
                  for m in range(KT)]

            # ---- input DMAs, spread across sequencers -------------------
            # (HWDGE only exists on SP + Activation; gpsimd uses SWDGE)
            # scalar: qk weights in consumption order (pair 0 first)
            nc.scalar.dma_start(wqk0a[:], wqk0a_e[:])
            nc.scalar.dma_start(wqk0b[:], wqk0b_e[:])
            nc.scalar.dma_start(wqkA[:], wqkA_e[:])
            nc.scalar.dma_start(wqkB[:], wqkB_e[:])
            # sync: activations in consumption order, then proj weights
            for k in range(KT):
                nc.sync.dma_start(xT[k][:], xT_e[:, k, :])
            nc.sync.dma_start(wp[:], wp_e[:])
            # gpsimd: biases + V weights
            nc.gpsimd.dma_start(ball[:], ball_e[:])
            nc.gpsimd.dma_start(wv[:], wv_e[:])
            nc.gpsimd.dma_start(bv[:], bv_e[:])

            # lhsT slice of [W_q*SCALE | W_k] for contraction chunk k, tile m
            def qk_lhsT(k, m):
                p = m % 6
                kcol = 128 if m >= 6 else 0
                if p == 0:
                    if k == 0:
                        return wqk0a[:, kcol:kcol + 128]
                    return wqk0b[:, 256 * (k - 1) + kcol:256 * (k - 1) + kcol + 128]
                if p <= 2:
                    o = 512 * k + 256 * (p - 1) + kcol
                    return wqkA[:, o:o + 128]
                o = 768 * k + 256 * (p - 3) + kcol
                return wqkB[:, o:o + 128]

            # ---- filler sub-chunks (1-bank PSUM each, ~6 MMs) -----------
            def sub_qk(m, j):
                def emit():
                    ps = f_pool.tile([128, 512], f32, name="fps", tag="fps")
                    for k in range(KT):
                        nc.tensor.matmul(
                            ps[:],
                            lhsT=qk_lhsT(k, m),
                            rhs=xT[k][:, 512 * j:512 * (j + 1)],
                            start=(k == 0), stop=(k == KT - 1),
                        )
                    nc.vector.tensor_scalar_add(
                        qkT[m][:, 512 * j:512 * (j + 1)], ps[:], ball[:, m:m + 1])
                return emit

            def sub_v(t, part):
                c0, cw = ((0, 512), (512, 256))[part]
                h0, hn = ((0, 8), (8, 4))[part]
                def emit():
                    ps = f_pool.tile([128, 512], f32, name="fps", tag="fps")
                    for k in range(KT):
                        nc.tensor.matmul(
                            ps[:, 0:cw],
                            lhsT=xT[k][:, 128 * t:128 * (t + 1)],
                            rhs=wv[:, C * k + c0:C * k + c0 + cw],
                            start=(k == 0), stop=(k == KT - 1),
                        )
                    if part == 0:
                        nc.gpsimd.memset(v_sb[t][:, :, D:D + 1], 1.0)
                    nc.vector.tensor_add(
                        v_sb[t][:, h0:h0 + hn, 0:D],
                        ps[:, 0:cw].rearrange("p (h x) -> p h x", x=D),
                        bv[:, h0:h0 + hn, :],
                    )
                return emit

            ph3_out = [[persist.tile([128, 512], f32, name=f"p3_{c}_{j}",
                                     tag=f"p3_{c}_{j}") for j in range(NJ)]
                       for c in range(KT)]

            def wp_lhsT(m, c):
                return wp[:, C * m + 128 * c:C * m + 128 * c + 128]

            def sub_ph3a(c, j):
                def emit():
                    ps = f_pool.tile([128, 512], f32, name="fps", tag="fps")
                    for m in range(4):
                        nc.tensor.matmul(
                            ps[:],
                            lhsT=wp_lhsT(m, c),
                            rhs=oT[m][:, 512 * j:512 * (j + 1)],
                            start=(m == 0), stop=(m == 3),
                        )
                    nc.vector.tensor_scalar_add(
                        ph3_out[c][j][:], ps[:], ball[:, 12 + c:13 + c])
                return emit

            def sub_ph3b(c, j):
                def emit():
                    ps = f_pool.tile([128, 512], f32, name="fps", tag="fps")
                    nc.tensor.matmul(
                        ps[:],
                        lhsT=wp_lhsT(4, c),
                        rhs=oT[4][:, 512 * j:512 * (j + 1)],
                        start=True, stop=True,
                    )
                    nc.vector.tensor_add(
                        ph3_out[c][j][:], ph3_out[c][j][:], ps[:])
                return emit

            # ---- head-pair machinery ------------------------------------
            def s_step(pair, m, e_e, e_o):
                """4 S matmuls alternating row-halves + 2 exps."""
                qt, kt = qkT[pair], qkT[6 + pair]
                t_e = s_pool.tile([128, N], f32, name="sps", tag="sps")
                t_o = s_pool.tile([128, N], f32, name="sps", tag="sps")
                for j in range(NJ):
                    for po, t in ((0, t_e), (64, t_o)):
                        nc.tensor.matmul(
                            t[:, 512 * j:512 * (j + 1)],
                            lhsT=kt[po:po + 64, 128 * m:128 * (m + 1)],
                            rhs=qt[po:po + 64, 512 * j:512 * (j + 1)],
                            start=True, stop=True,
                        )
                for t, lst in ((t_e, e_e), (t_o, e_o)):
                    e_sb = e_pool.tile([128, N], bf16, name="e_sb", tag="e_sb")
                    nc.scalar.activation(e_sb[:], t[:], Exp)
                    lst.append(e_sb)

            class HeadPV:
                """Trailing PV + normalize for one head, consumed task-wise."""
                def __init__(self, h, e_tiles):
                    self.h, self.e = h, e_tiles
                    self.m = 0
                    self.o_ps = [o_pool.tile([65, 512], f32, name="o_ps",
                                             tag="o_ps") for _ in range(NJ)]

                def step(self):
                    h, m = self.h, self.m
                    for j in range(NJ):
                        nc.tensor.matmul(
                            self.o_ps[j][:, :],
                            lhsT=v_sb[m][:, h, :],
                            rhs=self.e[m][:, 512 * j:512 * (j + 1)],
                            start=(m == 0), stop=(m == MT - 1),
                        )
                    self.m += 1
                    if self.m == MT:
                        self.finish()
                        return True
                    return False

                def finish(self):
                    # stage PSUM->SBUF first so the o accumulator banks free
                    # earlier; the normalize chain then runs off SBUF.
                    h, po = self.h, 64 * (self.h % 2)
                    sc = r_pool.tile([1, N], f32, name="sc", tag="sc")
                    st = st_pool.tile([64, N], f32, name="st", tag="st")
                    if h >= 10:
                        # ScalarE is idle once its exp stream ends: stage the
                        # last pair there, and run the whole normalize per-j
                        # so the j=0 slice of oT[5] lands earlier (the
                        # proj tail consumes j-outer).
                        for j in range(NJ):
                            jsl = slice(512 * j, 512 * (j + 1))
                            nc.scalar.activation(
                                sc[0:1, jsl], self.o_ps[j][64:65, :], CopyF)
                            nc.scalar.activation(
                                st[0:64, jsl], self.o_ps[j][0:64, :], CopyF)
                            r = r_pool.tile([1, 512], f32, name="r", tag="r")
                            nc.vector.reciprocal_approx_fast(
                                r[0:1, :], sc[0:1, jsl])
                            rb = rb_pool.tile([64, 512], f32, name="rb", tag="rb")
                            nc.gpsimd.partition_broadcast(rb[:], r[0:1, :])
                            nc.vector.tensor_mul(
                                oT[h // 2][po:po + 64, jsl],
                                st[0:64, jsl], rb[0:64, :])
                        return
                    for j in range(NJ):
                        nc.vector.tensor_copy(
                            sc[0:1, 512 * j:512 * (j + 1)], self.o_ps[j][64:65, :])
                        nc.vector.tensor_copy(
                            st[0:64, 512 * j:512 * (j + 1)], self.o_ps[j][0:64, :])
                    r = r_pool.tile([1, N], f32, name="r", tag="r")
                    nc.vector.reciprocal_approx_fast(r[0:1, :], sc[0:1, :])
                    rb = rb_pool.tile([64, N], f32, name="rb", tag="rb")
                    nc.gpsimd.partition_broadcast(rb[:], r[0:1, :])
                    for j in range(NJ):
                        nc.vector.tensor_mul(
                            oT[h // 2][po:po + 64, 512 * j:512 * (j + 1)],
                            st[0:64, 512 * j:512 * (j + 1)],
                            rb[0:64, 512 * j:512 * (j + 1)],
                        )

            # ---- the software-pipelined schedule ------------------------
            from collections import deque
            fillers = deque()
            pv_queue = deque()   # HeadPV objects, strictly ordered

            def drain_pv(max_tasks):
                n = 0
                while pv_queue and n < max_tasks:
                    hp = pv_queue[0]
                    if hp.m >= len(hp.e):
                        break  # exp for this m not emitted yet
                    if hp.step():
                        pv_queue.popleft()
                    n += 1

            def drain_fillers(max_chunks):
                for _ in range(min(max_chunks, len(fillers))):
                    fillers.popleft()()

            # proj partial chunks, consumed strictly in order; stage 'a'
            # (m=0..3) unlocks when heads 0-7 retired, 'b' (m=4) when heads
            # 8-9 retired.  The last PROJ_RESERVE stay for the drain window.
            proj_pend = deque(
                [("a", sub_ph3a(c, j)) for c in range(KT) for j in range(NJ)] +
                [("b", sub_ph3b(c, j)) for c in range(KT) for j in range(NJ)])
            PROJ_RESERVE = 6

            def proj_gate_ok():
                if not proj_pend:
                    return False
                stage = proj_pend[0][0]
                hmax = 7 if stage == "a" else 9
                return not any(hp.h <= hmax for hp in pv_queue)

            # prelude: QK chunks for pair 0 (PE warmup, un-gated)
            for j in range(NJ):
                sub_qk(0, j)()
            for j in range(NJ):
                sub_qk(6, j)()
            # fillers for pair 0: all V sub-chunks, then pair-1 QK chunks
            for t in range(MT):
                fillers.append(sub_v(t, 0))
                fillers.append(sub_v(t, 1))
            for m in (1, 7):
                for j in range(NJ):
                    fillers.append(sub_qk(m, j))

            for pair in range(6):
                e_e, e_o = [], []
                pend_e, pend_o = HeadPV(2 * pair, e_e), HeadPV(2 * pair + 1, e_o)
                if pair < 4:
                    new_fill = [sub_qk(pair + 2, j) for j in range(NJ)] + \
                               [sub_qk(6 + pair + 2, j) for j in range(NJ)]
                else:
                    new_fill = []
                for m in range(MT):
                    s_step(pair, m, e_e, e_o)
                    if m == 2:
                        pv_queue.append(pend_e)
                        pv_queue.append(pend_o)
                    drain_pv(2)
                    nfill = 3 if pair == 0 else 1
                    drain_fillers(nfill)
                    if new_fill and m % 4 == 1:
                        fillers.append(new_fill.pop(0))
                        fillers.append(new_fill.pop(0))
                    # once earlier pairs fully retire, feed proj partials as
                    # fillers for the otherwise filler-less pairs 4/5 — but
                    # hold PROJ_RESERVE back for the final PV/normalize drain.
                    if (pair >= 4 and len(proj_pend) > PROJ_RESERVE
                            and proj_gate_ok()):
                        fillers.append(proj_pend.popleft()[1])
            # drain what remains, interleaving the reserved proj chunks
            while pv_queue:
                drain_pv(4)
                if proj_gate_ok():
                    proj_pend.popleft()[1]()
                else:
                    drain_fillers(1)
            drain_fillers(len(fillers))
            while proj_pend:
                proj_pend.popleft()[1]()

            # ---- phase 3 tail: add the m=5 contribution + DMA out -------
            dma_engines = [nc.sync, nc.scalar, nc.gpsimd]
            for j in range(NJ):
                for c in range(KT):
                    ps = f_pool.tile([128, 512], f32, name="fps", tag="fps")
                    nc.tensor.matmul(
                        ps[:],
                        lhsT=wp_lhsT(5, c),
                        rhs=oT[5][:, 512 * j:512 * (j + 1)],
                        start=True, stop=True,
                    )
                    oc = out_pool.tile([128, 512], bf16, name="oc", tag="oc")
                    nc.vector.tensor_add(oc[:], ps[:], ph3_out[c][j][:])
                    eng = dma_engines[(j * KT + c) % 3]
                    eng.dma_start(
                        out_e[128 * c:128 * (c + 1), 512 * j:512 * (j + 1)], oc[:])

    nc.compile()
    return nc


def prep_inputs(x, W_qkv, b_qkv, W_proj, b_proj):
    """Host-side shard + layout prep. Returns in_maps for 8 cores."""
    x = np.asarray(x, dtype=np.float32)
    W_qkv = np.asarray(W_qkv, dtype=np.float32)
    b_qkv = np.asarray(b_qkv, dtype=np.float32)
    W_proj = np.asarray(W_proj, dtype=np.float32)
    b_proj = np.asarray(b_proj, dtype=np.float32)

    w_qk = np.concatenate([W_qkv[:, :C] * SCALE, W_qkv[:, C:2 * C]], axis=1)
    w_qk = np.ascontiguousarray(w_qk).astype(BF16)      # [768, 1536]

    def kmajor(w):  # [768, F] -> [128, KT*F]
        f = w.shape[1]
        return np.ascontiguousarray(
            w.reshape(KT, 128, f).transpose(1, 0, 2).reshape(128, KT * f))

    # pair-major column grouping: pair p holds [Q_p (128) | K_p (128)]
    pairs = [np.concatenate([w_qk[:, 128 * p:128 * (p + 1)],
                             w_qk[:, 768 + 128 * p:768 + 128 * (p + 1)]],
                            axis=1) for p in range(6)]       # each [768, 256]
    wqk0 = kmajor(pairs[0])                                  # [128, 1536]
    wqk0a = np.ascontiguousarray(wqk0[:, :256])              # k=0
    wqk0b = np.ascontiguousarray(wqk0[:, 256:])              # k=1..5
    wqkA = kmajor(np.concatenate(pairs[1:3], axis=1))        # [128, 3072]
    wqkB = kmajor(np.concatenate(pairs[3:6], axis=1))        # [128, 4608]

    w_v = kmajor(np.ascontiguousarray(W_qkv[:, 2 * C:]).astype(BF16))
    w_p = kmajor(W_proj.astype(BF16))

    b_qk = np.concatenate([b_qkv[:C] * SCALE, b_qkv[C:2 * C]])
    b_all = np.empty((128, 18), np.float32)
    b_all[:, :12] = b_qk.reshape(12, 128).T
    b_all[:, 12:] = b_proj.reshape(6, 128).T
    b_v = np.ascontiguousarray(
        np.broadcast_to(b_qkv[2 * C:].reshape(H, D), (128, H, D))).astype(np.float32)

    shared = {"wqk0a": wqk0a, "wqk0b": wqk0b, "wqkA": wqkA, "wqkB": wqkB,
              "w_v": w_v, "w_proj": w_p, "b_all": b_all, "b_v": b_v}
    in_maps = []
    for b in range(NCORES):
        xT = np.ascontiguousarray(x[b].T).astype(BF16)       # [768, 1024]
        m = dict(shared)
        m["xT"] = np.ascontiguousarray(
            xT.reshape(KT, 128, N).transpose(1, 0, 2))       # [128, 6, 1024]
        in_maps.append(m)
    return in_maps


def kernel(x, W_qkv, b_qkv, W_proj, b_proj):
    from concourse.bass_utils import run_bass_kernel_spmd

    nc = _CACHE.get("nc")
    if nc is None:
        nc = _CACHE["nc"] = build_nc()

    in_maps = prep_inputs(x, W_qkv, b_qkv, W_proj, b_proj)
    res = run_bass_kernel_spmd(nc, in_maps, core_ids=list(range(NCORES)))
    out = np.empty((B, N, C), np.float32)
    for b in range(NCORES):
        out[b] = res.results[b]["outT"].astype(np.float32).T
    return out


# revision 13
# speedup vs baseline: 1.0308x; 1.0308x over previous
"""Trainium2 Bass kernel for batch-8 multi-head attention.

Strategy: pure data parallelism — one batch element per NeuronCore (B=8,
8 cores), zero collectives.  All inputs are pre-arranged on the host so the
device kernel only ever runs dense matmuls in its preferred layouts.

  per-core DRAM inputs (bf16 unless noted):
    xT        [128, 6, 1024]  x[b].T k-chunked       (feature-major activations)
    wqk0a     [128, 256]      pair-0 (m=0,6) cols of [W_q*SCALE | W_k], k=0
    wqk0b     [128, 1280]     pair-0 cols, k=1..5
    wqkA      [128, 3072]     pairs 1-2 cols, k-major
    wqkB      [128, 4608]     pairs 3-5 cols, k-major
    wv        [128, 4608]     W_v k-major
    wp        [128, 4608]     W_proj k-major
    b_all     [128, 18] f32   per-partition bias chunks (12 qk + 6 proj)
    b_v       [128, 12, 64] f32  V bias broadcast along partitions
  output:
    outT      [768, 1024] bf16  (attention output)^T — host transposes back

Device pipeline per core (program order interleaves phases so ScalarE's exp
stream starts early and the PE never starves — PE occupancy is the
bottleneck: ~165us of matmul on a ~143.5us column-count floor):
  QK^T = w_qk^T @ xT                 -> 12 tiles [128, 1024], 2 heads/tile
  V    = xT^T @ w_v + b_v            -> 8 tiles [128, 12, 65], ones col fused
  per head h:
    S^T[m]   = K_h @ Q_h^T           (K=64 contraction)
    expS     = exp(S^T)  on ScalarE  (no max subtraction: |logits| < ~8)
    [O^T|s]  = [V_m|1]^T @ expS      (PSUM accumulate over m; row 64 = sums)
    oT       = O^T * (1/s)           (recip_approx + partition-broadcast + mul)
  outT = w_proj^T @ oT + b_proj      (partials m0-3 early, m4 after heads 8/9,
                                      m5-only tail after the last normalize)

DMA plan: input dma_starts are spread across the scalar/sync/vector/gpsimd
sequencers (one DIRECT2D is ~610ns of sequencer time — serializing 26 of
them on SP delayed PE saturation to ~25us).  Output chunks round-robin over
all four sequencers as they complete.

PSUM: s-tiles 2-bank x2 + o-accumulators 1-bank x2 + filler chunks 1-bank x2
= 8 banks, shared across all phases with no pool barrier.

Attempts that CORRUPT on hardware (sim-clean, do not retry): genuinely
overlapping 64-row-tiled S matmuls with neighboring 128-mode matmuls
(tile_position + early PSUM release), [65,512] DVE copies from PSUM, and
reciprocal/mul reading o_ps PSUM directly.  Finer [128,512] exps are
correct but slower (+287ns fixed cost per activation instruction).
"""

import os
import sys

os.environ.setdefault("BASS_PERFETTO_PROFILE_ALL_CORES", "1")
if "/opt/trn_rl_repo" not in sys.path:
    sys.path.insert(0, "/opt/trn_rl_repo")

import numpy as np
import ml_dtypes

B, N, C, H = 8, 1024, 768, 12
D = C // H                # 64 head dim
SCALE = D ** -0.5
NCORES = 8
KT = C // 128             # 6 contraction tiles over C
MT = N // 128             # 8 token blocks
NJ = N // 512             # 2 query chunks of 512
BF16 = ml_dtypes.bfloat16

_CACHE = {}


def build_nc():
    """Build + compile the per-core Bass graph (identical on all 8 cores)."""
    import concourse.tile as tile
    from concourse import bacc, mybir

    f32 = mybir.dt.float32
    bf16 = mybir.dt.bfloat16
    Exp = mybir.ActivationFunctionType.Exp
    CopyF = mybir.ActivationFunctionType.Copy

    nc = bacc.Bacc("TRN2", target_bir_lowering=False, debug=False,
                   num_devices=NCORES)

    xT_e = nc.dram_tensor("xT", [128, KT, N], bf16, kind="ExternalInput").ap()
    wqk0a_e = nc.dram_tensor("wqk0a", [128, 256], bf16, kind="ExternalInput").ap()
    wqk0b_e = nc.dram_tensor("wqk0b", [128, 1280], bf16, kind="ExternalInput").ap()
    wqkA_e = nc.dram_tensor("wqkA", [128, KT * 512], bf16, kind="ExternalInput").ap()
    wqkB_e = nc.dram_tensor("wqkB", [128, KT * 768], bf16, kind="ExternalInput").ap()
    wv_e = nc.dram_tensor("w_v", [128, KT * C], bf16, kind="ExternalInput").ap()
    wp_e = nc.dram_tensor("w_proj", [128, KT * C], bf16, kind="ExternalInput").ap()
    ball_e = nc.dram_tensor("b_all", [128, 18], f32, kind="ExternalInput").ap()
    bv_e = nc.dram_tensor("b_v", [128, H, D], f32, kind="ExternalInput").ap()
    out_e = nc.dram_tensor("outT", [C, N], bf16, kind="ExternalOutput").ap()

    with tile.TileContext(nc) as tc:
        from contextlib import ExitStack

        with ExitStack() as es:
            persist = es.enter_context(tc.tile_pool(name="persist", bufs=1))
            s_pool = es.enter_context(tc.tile_pool(name="spsum", bufs=2, space="PSUM"))
            o_pool = es.enter_context(tc.tile_pool(name="opsum", bufs=2, space="PSUM"))
            f_pool = es.enter_context(tc.tile_pool(name="fpsum", bufs=2, space="PSUM"))
            e_pool = es.enter_context(tc.tile_pool(name="expS", bufs=14))
            r_pool = es.enter_context(tc.tile_pool(name="recip", bufs=2))
            st_pool = es.enter_context(tc.tile_pool(name="stage", bufs=3))
            rb_pool = es.enter_context(tc.tile_pool(name="recipb", bufs=2))
            out_pool = es.enter_context(tc.tile_pool(name="outc", bufs=3))

            # ---- persistent SBUF tiles ----------------------------------
            xT = [persist.tile([128, N], bf16, name=f"xT{k}", tag=f"xT{k}")
                  for k in range(KT)]
            wqk0a = persist.tile([128, 256], bf16, name="wqk0a", tag="wqk0a")
            wqk0b = persist.tile([128, 1280], bf16, name="wqk0b", tag="wqk0b")
            wqkA = persist.tile([128, KT * 512], bf16, name="wqkA", tag="wqkA")
            wqkB = persist.tile([128, KT * 768], bf16, name="wqkB", tag="wqkB")
            wv = persist.tile([128, KT * C], bf16, name="wv", tag="wv")
            wp = persist.tile([128, KT * C], bf16, name="wp", tag="wp")
            ball = persist.tile([128, 18], f32, name="ball", tag="ball")
            bv = persist.tile([128, H, D], f32, name="bv", tag="bv")
            ones64 = persist.tile([1, 64], f32, name="ones64", tag="ones64")
            qkT = [persist.tile([128, N], bf16, name=f"qkT{m}", tag=f"qkT{m}")
                   for m in range(12)]
            v_sb = [persist.tile([128, H, D + 1], bf16, name=f"v{t}", tag=f"v{t}")
                    for t in range(MT)]
            oT = [persist.tile([128, N], bf16, name=f"oT{m}", tag=f"oT{m}")
                  for m in range(KT)]

            # ---- input DMAs, spread across sequencers -------------------
            # (HWDGE only exists on SP + Activation; gpsimd uses SWDGE.)
            # The 16 DMA queues are FIFO and shared: a big transfer issued
            # early blocks later-issued critical ones, so everything large
            # queues BEHIND the xT chunks the prelude needs first.
            nc.scalar.dma_start(wqk0a[:], wqk0a_e[:])
            nc.scalar.dma_start(wqk0b[:], wqk0b_e[:])
            for k in range(KT):
                nc.sync.dma_start(xT[k][:], xT_e[:, k, :])
            nc.sync.dma_start(wqkA[:], wqkA_e[:])
            nc.sync.dma_start(wqkB[:], wqkB_e[:])
            nc.sync.dma_start(wp[:], wp_e[:])
            # gpsimd: biases + V weights
            nc.gpsimd.dma_start(ball[:], ball_e[:])
            nc.gpsimd.dma_start(wv[:], wv_e[:])
            nc.gpsimd.dma_start(bv[:], bv_e[:])
            nc.gpsimd.memset(ones64[:], 1.0)

            # lhsT slice of [W_q*SCALE | W_k] for contraction chunk k, tile m
            def qk_lhsT(k, m):
                p = m % 6
                kcol = 128 if m >= 6 else 0
                if p == 0:
                    if k == 0:
                        return wqk0a[:, kcol:kcol + 128]
                    return wqk0b[:, 256 * (k - 1) + kcol:256 * (k - 1) + kcol + 128]
                if p <= 2:
                    o = 512 * k + 256 * (p - 1) + kcol
                    return wqkA[:, o:o + 128]
                o = 768 * k + 256 * (p - 3) + kcol
                return wqkB[:, o:o + 128]

            # ---- filler sub-chunks (1-bank PSUM each, ~6 MMs) -----------
            def sub_qk(m, j):
                def emit():
                    ps = f_pool.tile([128, 512], f32, name="fps", tag="fps")
                    for k in range(KT):
                        nc.tensor.matmul(
                            ps[:],
                            lhsT=qk_lhsT(k, m),
                            rhs=xT[k][:, 512 * j:512 * (j + 1)],
                            start=(k == 0), stop=(k == KT - 1),
                        )
                    nc.vector.tensor_scalar_add(
                        qkT[m][:, 512 * j:512 * (j + 1)], ps[:], ball[:, m:m + 1])
                return emit

            def sub_v(t, part):
                c0, cw = ((0, 512), (512, 256))[part]
                h0, hn = ((0, 8), (8, 4))[part]
                def emit():
                    ps = f_pool.tile([128, 512], f32, name="fps", tag="fps")
                    for k in range(KT):
                        nc.tensor.matmul(
                            ps[:, 0:cw],
                            lhsT=xT[k][:, 128 * t:128 * (t + 1)],
                            rhs=wv[:, C * k + c0:C * k + c0 + cw],
                            start=(k == 0), stop=(k == KT - 1),
                        )
                    if part == 0:
                        nc.gpsimd.memset(v_sb[t][:, :, D:D + 1], 1.0)
                    nc.vector.tensor_add(
                        v_sb[t][:, h0:h0 + hn, 0:D],
                        ps[:, 0:cw].rearrange("p (h x) -> p h x", x=D),
                        bv[:, h0:h0 + hn, :],
                    )
                return emit

            ph3_out = [[persist.tile([128, 512], f32, name=f"p3_{c}_{j}",
                                     tag=f"p3_{c}_{j}") for j in range(NJ)]
                       for c in range(KT)]

            def wp_lhsT(m, c):
                return wp[:, C * m + 128 * c:C * m + 128 * c + 128]

            def sub_ph3a(c, j):
                def emit():
                    ps = f_pool.tile([128, 512], f32, name="fps", tag="fps")
                    for m in range(4):
                        nc.tensor.matmul(
                            ps[:],
                            lhsT=wp_lhsT(m, c),
                            rhs=oT[m][:, 512 * j:512 * (j + 1)],
                            start=(m == 0), stop=(m == 3),
                        )
                    nc.vector.tensor_scalar_add(
                        ph3_out[c][j][:], ps[:], ball[:, 12 + c:13 + c])
                return emit

            def sub_ph3b(c, j):
                def emit():
                    ps = f_pool.tile([128, 512], f32, name="fps", tag="fps")
                    nc.tensor.matmul(
                        ps[:],
                        lhsT=wp_lhsT(4, c),
                        rhs=oT[4][:, 512 * j:512 * (j + 1)],
                        start=True, stop=True,
                    )
                    nc.vector.tensor_add(
                        ph3_out[c][j][:], ph3_out[c][j][:], ps[:])
                return emit

            # ---- head-pair machinery ------------------------------------
            def s_step(pair, m, e_e, e_o):
                """4 S matmuls alternating row-halves + 2 exps."""
                qt, kt = qkT[pair], qkT[6 + pair]
                t_e = s_pool.tile([128, N], f32, name="sps", tag="sps")
                t_o = s_pool.tile([128, N], f32, name="sps", tag="sps")
                for j in range(NJ):
                    for po, t in ((0, t_e), (64, t_o)):
                        nc.tensor.matmul(
                            t[:, 512 * j:512 * (j + 1)],
                            lhsT=kt[po:po + 64, 128 * m:128 * (m + 1)],
                            rhs=qt[po:po + 64, 512 * j:512 * (j + 1)],
                            start=True, stop=True,
                        )
                for t, lst in ((t_e, e_e), (t_o, e_o)):
                    e_sb = e_pool.tile([128, N], bf16, name="e_sb", tag="e_sb")
                    nc.scalar.activation(e_sb[:], t[:], Exp)
                    lst.append(e_sb)

            class HeadPV:
                """Trailing PV + normalize for one head, consumed task-wise."""
                def __init__(self, h, e_tiles):
                    self.h, self.e = h, e_tiles
                    self.m = 0
                    self.o_ps = [o_pool.tile([65, 512], f32, name="o_ps",
                                             tag="o_ps") for _ in range(NJ)]

                def step(self):
                    h, m = self.h, self.m
                    for j in range(NJ):
                        nc.tensor.matmul(
                            self.o_ps[j][:, :],
                            lhsT=v_sb[m][:, h, :],
                            rhs=self.e[m][:, 512 * j:512 * (j + 1)],
                            start=(m == 0), stop=(m == MT - 1),
                        )
                    self.m += 1
                    if self.m == MT:
                        self.finish()
                        return True
                    return False

                def finish(self):
                    # stage PSUM->SBUF first so the o accumulator banks free
                    # earlier; the normalize chain then runs off SBUF.
                    h, po = self.h, 64 * (self.h % 2)
                    sc = r_pool.tile([1, N], f32, name="sc", tag="sc")
                    st = st_pool.tile([64, N], f32, name="st", tag="st")
                    if h >= 10:
                        # ScalarE is idle once its exp stream ends: stage the
                        # last pair there, and run the whole normalize per-j
                        # so the j=0 slice of oT[5] lands earlier (the
                        # proj tail consumes j-outer).  The partition
                        # broadcast runs as a tiny PE matmul (ones^T @ r,
                        # 216ns) instead of GpSimd (~1us) — latency is what
                        # matters on this chain.
                        for j in range(NJ):
                            jsl = slice(512 * j, 512 * (j + 1))
                            nc.scalar.activation(
                                sc[0:1, jsl], self.o_ps[j][64:65, :], CopyF)
                            nc.scalar.activation(
                                st[0:64, jsl], self.o_ps[j][0:64, :], CopyF)
                            r = r_pool.tile([1, 512], f32, name="r", tag="r")
                            nc.vector.reciprocal_approx_fast(
                                r[0:1, :], sc[0:1, jsl])
                            rb = f_pool.tile([64, 512], f32, name="rbp",
                                             tag="fps")
                            nc.tensor.matmul(
                                rb[:, :], lhsT=ones64[0:1, :], rhs=r[0:1, :],
                                start=True, stop=True)
                            nc.vector.tensor_mul(
                                oT[h // 2][po:po + 64, jsl],
                                st[0:64, jsl], rb[0:64, :])
                        return
                    for j in range(NJ):
                        nc.vector.tensor_copy(
                            sc[0:1, 512 * j:512 * (j + 1)], self.o_ps[j][64:65, :])
                        nc.vector.tensor_copy(
                            st[0:64, 512 * j:512 * (j + 1)], self.o_ps[j][0:64, :])
                    r = r_pool.tile([1, N], f32, name="r", tag="r")
                    nc.vector.reciprocal_approx_fast(r[0:1, :], sc[0:1, :])
                    rb = rb_pool.tile([64, N], f32, name="rb", tag="rb")
                    nc.gpsimd.partition_broadcast(rb[:], r[0:1, :])
                    for j in range(NJ):
                        nc.vector.tensor_mul(
                            oT[h // 2][po:po + 64, 512 * j:512 * (j + 1)],
                            st[0:64, 512 * j:512 * (j + 1)],
                            rb[0:64, 512 * j:512 * (j + 1)],
                        )

            # ---- the software-pipelined schedule ------------------------
            from collections import deque
            fillers = deque()
            pv_queue = deque()   # HeadPV objects, strictly ordered

            def drain_pv(max_tasks):
                n = 0
                while pv_queue and n < max_tasks:
                    hp = pv_queue[0]
                    if hp.m >= len(hp.e):
                        break  # exp for this m not emitted yet
                    if hp.step():
                        pv_queue.popleft()
                    n += 1

            def drain_fillers(max_chunks):
                for _ in range(min(max_chunks, len(fillers))):
                    fillers.popleft()()

            # proj partial chunks, emitted strictly in order (the 'b' m=4
            # add must follow the 'a' m=0-3 init of the same chunk); stage
            # 'a' unlocks when heads 0-7 retired, 'b' when heads 8-9
            # retired.  Never queued into `fillers` so ordering holds.
            proj_pend = deque(
                [("a", sub_ph3a(c, j)) for c in range(KT) for j in range(NJ)] +
                [("b", sub_ph3b(c, j)) for c in range(KT) for j in range(NJ)])

            def proj_gate_ok():
                if not proj_pend:
                    return False
                stage = proj_pend[0][0]
                hmax = 7 if stage == "a" else 9
                return not any(hp.h <= hmax for hp in pv_queue)

            # prelude: QK chunks for pair 0 (PE warmup, un-gated)
            for j in range(NJ):
                sub_qk(0, j)()
            for j in range(NJ):
                sub_qk(6, j)()
            # fillers for pair 0: all V sub-chunks, then pair-1 QK chunks
            for t in range(MT):
                fillers.append(sub_v(t, 0))
                fillers.append(sub_v(t, 1))
            for m in (1, 7):
                for j in range(NJ):
                    fillers.append(sub_qk(m, j))

            for pair in range(6):
                e_e, e_o = [], []
                pend_e, pend_o = HeadPV(2 * pair, e_e), HeadPV(2 * pair + 1, e_o)
                if pair < 4:
                    new_fill = [sub_qk(pair + 2, j) for j in range(NJ)] + \
                               [sub_qk(6 + pair + 2, j) for j in range(NJ)]
                else:
                    new_fill = []
                for m in range(MT):
                    s_step(pair, m, e_e, e_o)
                    if m == 2:
                        pv_queue.append(pend_e)
                        pv_queue.append(pend_o)
                    drain_pv(2)
                    nfill = 3 if pair == 0 else 1
                    drain_fillers(nfill)
                    if new_fill and m % 4 == 1:
                        fillers.append(new_fill.pop(0))
                        fillers.append(new_fill.pop(0))
                    # once earlier pairs fully retire, feed proj partials to
                    # the otherwise filler-less pairs 4/5 (emitted directly,
                    # keeping the a->b order per chunk); pair 5 takes two so
                    # the cheap single-MM 'b' chunks clear before the drain.
                    if pair >= 4 and proj_gate_ok():
                        proj_pend.popleft()[1]()
                        if pair == 5 and proj_gate_ok():
                            proj_pend.popleft()[1]()
            # drain what remains, interleaving the remaining proj chunks
            while pv_queue:
                drain_pv(2)
                for _ in range(2):
                    if proj_gate_ok():
                        proj_pend.popleft()[1]()
                drain_fillers(1)
            drain_fillers(len(fillers))
            while proj_pend:
                proj_pend.popleft()[1]()

            # ---- phase 3 tail: add the m=5 contribution + DMA out -------
            dma_engines = [nc.gpsimd, nc.sync, nc.scalar]
            for j in range(NJ):
                for c in range(KT):
                    ps = f_pool.tile([128, 512], f32, name="fps", tag="fps")
                    nc.tensor.matmul(
                        ps[:],
                        lhsT=wp_lhsT(5, c),
                        rhs=oT[5][:, 512 * j:512 * (j + 1)],
                        start=True, stop=True,
                    )
                    oc = out_pool.tile([128, 512], bf16, name="oc", tag="oc")
                    nc.vector.tensor_add(oc[:], ps[:], ph3_out[c][j][:])
                    eng = dma_engines[(j * KT + c) % 3]
                    eng.dma_start(
                        out_e[128 * c:128 * (c + 1), 512 * j:512 * (j + 1)], oc[:])

    nc.compile()
    return nc


def prep_inputs(x, W_qkv, b_qkv, W_proj, b_proj):
    """Host-side shard + layout prep. Returns in_maps for 8 cores."""
    x = np.asarray(x, dtype=np.float32)
    W_qkv = np.asarray(W_qkv, dtype=np.float32)
    b_qkv = np.asarray(b_qkv, dtype=np.float32)
    W_proj = np.asarray(W_proj, dtype=np.float32)
    b_proj = np.asarray(b_proj, dtype=np.float32)

    w_qk = np.concatenate([W_qkv[:, :C] * SCALE, W_qkv[:, C:2 * C]], axis=1)
    w_qk = np.ascontiguousarray(w_qk).astype(BF16)      # [768, 1536]

    def kmajor(w):  # [768, F] -> [128, KT*F]
        f = w.shape[1]
        return np.ascontiguousarray(
            w.reshape(KT, 128, f).transpose(1, 0, 2).reshape(128, KT * f))

    # pair-major column grouping: pair p holds [Q_p (128) | K_p (128)]
    pairs = [np.concatenate([w_qk[:, 128 * p:128 * (p + 1)],
                             w_qk[:, 768 + 128 * p:768 + 128 * (p + 1)]],
                            axis=1) for p in range(6)]       # each [768, 256]
    wqk0 = kmajor(pairs[0])                                  # [128, 1536]
    wqk0a = np.ascontiguousarray(wqk0[:, :256])              # k=0
    wqk0b = np.ascontiguousarray(wqk0[:, 256:])              # k=1..5
    wqkA = kmajor(np.concatenate(pairs[1:3], axis=1))        # [128, 3072]
    wqkB = kmajor(np.concatenate(pairs[3:6], axis=1))        # [128, 4608]

    w_v = kmajor(np.ascontiguousarray(W_qkv[:, 2 * C:]).astype(BF16))
    w_p = kmajor(W_proj.astype(BF16))

    b_qk = np.concatenate([b_qkv[:C] * SCALE, b_qkv[C:2 * C]])
    b_all = np.empty((128, 18), np.float32)
    b_all[:, :12] = b_qk.reshape(12, 128).T
    b_all[:, 12:] = b_proj.reshape(6, 128).T
    b_v = np.ascontiguousarray(
        np.broadcast_to(b_qkv[2 * C:].reshape(H, D), (128, H, D))).astype(np.float32)

    shared = {"wqk0a": wqk0a, "wqk0b": wqk0b, "wqkA": wqkA, "wqkB": wqkB,
              "w_v": w_v, "w_proj": w_p, "b_all": b_all, "b_v": b_v}
    in_maps = []
    for b in range(NCORES):
        xT = np.ascontiguousarray(x[b].T).astype(BF16)       # [768, 1024]
        m = dict(shared)
        m["xT"] = np.ascontiguousarray(
            xT.reshape(KT, 128, N).transpose(1, 0, 2))       # [128, 6, 1024]
        in_maps.append(m)
    return in_maps


def kernel(x, W_qkv, b_qkv, W_proj, b_proj):
    from concourse.bass_utils import run_bass_kernel_spmd

    nc = _CACHE.get("nc")
    if nc is None:
        nc = _CACHE["nc"] = build_nc()

    in_maps = prep_inputs(x, W_qkv, b_qkv, W_proj, b_proj)
    res = run_bass_kernel_spmd(nc, in_maps, core_ids=list(range(NCORES)))
    out = np.empty((B, N, C), np.float32)
    for b in range(NCORES):
        out[b] = res.results[b]["outT"].astype(np.float32).T
    return out


# revision 15
# speedup vs baseline: 1.0442x; 1.0130x over previous
"""Trainium2 Bass kernel for batch-8 multi-head attention.

Strategy: pure data parallelism — one batch element per NeuronCore (B=8,
8 cores), zero collectives.  All inputs are pre-arranged on the host so the
device kernel only ever runs dense matmuls in its preferred layouts.

  per-core DRAM inputs (bf16 unless noted):
    xT        [128, 6, 1024]  x[b].T k-chunked       (feature-major activations)
    wqk0a     [128, 256]      pair-0 (m=0,6) cols of [W_q*SCALE | W_k], k=0
    wqk0b     [128, 1280]     pair-0 cols, k=1..5
    wqkA      [128, 3072]     pairs 1-2 cols, k-major
    wqkB      [128, 4608]     pairs 3-5 cols, k-major
    wv        [128, 4608]     W_v k-major
    wp        [128, 4608]     W_proj k-major
    b_all     [128, 18] f32   per-partition bias chunks (12 qk + 6 proj)
    b_v       [128, 12, 64] f32  V bias broadcast along partitions
  output:
    outT      [768, 1024] bf16  (attention output)^T — host transposes back

Device pipeline per core (program order interleaves phases so ScalarE's exp
stream starts early and the PE never starves — PE occupancy is the
bottleneck: ~165us of matmul on a ~143.5us column-count floor):
  QK^T = w_qk^T @ xT                 -> 12 tiles [128, 1024], 2 heads/tile
  V    = xT^T @ w_v + b_v            -> 8 tiles [128, 12, 65], ones col fused
  per head h:
    S^T[m]   = K_h @ Q_h^T           (K=64 contraction)
    expS     = exp(S^T)  on ScalarE  (no max subtraction: |logits| < ~8)
    [O^T|s]  = [V_m|1]^T @ expS      (PSUM accumulate over m; row 64 = sums)
    oT       = O^T * (1/s)           (recip_approx + partition-broadcast + mul)
  outT = w_proj^T @ oT + b_proj      (partials m0-3 early, m4 after heads 8/9,
                                      m5-only tail after the last normalize)

DMA plan: input dma_starts are spread across the scalar/sync/vector/gpsimd
sequencers (one DIRECT2D is ~610ns of sequencer time — serializing 26 of
them on SP delayed PE saturation to ~25us).  Output chunks round-robin over
all four sequencers as they complete.

PSUM: s-tiles 2-bank x2 + o-accumulators 1-bank x2 + filler chunks 1-bank x2
= 8 banks, shared across all phases with no pool barrier.

Attempts that CORRUPT on hardware (sim-clean, do not retry): genuinely
overlapping 64-row-tiled S matmuls with neighboring 128-mode matmuls
(tile_position + early PSUM release), [65,512] DVE copies from PSUM, and
reciprocal/mul reading o_ps PSUM directly.  Finer [128,512] exps are
correct but slower (+287ns fixed cost per activation instruction).
"""

import os
import sys

os.environ.setdefault("BASS_PERFETTO_PROFILE_ALL_CORES", "1")
if "/opt/trn_rl_repo" not in sys.path:
    sys.path.insert(0, "/opt/trn_rl_repo")

import numpy as np
import ml_dtypes

B, N, C, H = 8, 1024, 768, 12
D = C // H                # 64 head dim
SCALE = D ** -0.5
NCORES = 8
KT = C // 128             # 6 contraction tiles over C
MT = N // 128             # 8 token blocks
NJ = N // 512             # 2 query chunks of 512
BF16 = ml_dtypes.bfloat16

_CACHE = {}


def build_nc():
    """Build + compile the per-core Bass graph (identical on all 8 cores)."""
    import concourse.tile as tile
    from concourse import bacc, mybir

    f32 = mybir.dt.float32
    bf16 = mybir.dt.bfloat16
    Exp = mybir.ActivationFunctionType.Exp
    CopyF = mybir.ActivationFunctionType.Copy

    nc = bacc.Bacc("TRN2", target_bir_lowering=False, debug=False,
                   num_devices=NCORES)

    xT_e = nc.dram_tensor("xT", [128, KT, N], bf16, kind="ExternalInput").ap()
    wqk0a_e = nc.dram_tensor("wqk0a", [128, 256], bf16, kind="ExternalInput").ap()
    wqk0b_e = nc.dram_tensor("wqk0b", [128, 1280], bf16, kind="ExternalInput").ap()
    wqkA_e = nc.dram_tensor("wqkA", [128, KT * 512], bf16, kind="ExternalInput").ap()
    wqkB_e = nc.dram_tensor("wqkB", [128, KT * 768], bf16, kind="ExternalInput").ap()
    wv_e = nc.dram_tensor("w_v", [128, KT * C], bf16, kind="ExternalInput").ap()
    wp_e = nc.dram_tensor("w_proj", [128, KT * C], bf16, kind="ExternalInput").ap()
    ball_e = nc.dram_tensor("b_all", [128, 18], f32, kind="ExternalInput").ap()
    bv_e = nc.dram_tensor("b_v", [128, H, D], f32, kind="ExternalInput").ap()
    out_e = nc.dram_tensor("outT", [C, N], bf16, kind="ExternalOutput").ap()

    with tile.TileContext(nc) as tc:
        from contextlib import ExitStack

        with ExitStack() as es:
            persist = es.enter_context(tc.tile_pool(name="persist", bufs=1))
            s_pool = es.enter_context(tc.tile_pool(name="spsum", bufs=2, space="PSUM"))
            o_pool = es.enter_context(tc.tile_pool(name="opsum", bufs=2, space="PSUM"))
            f_pool = es.enter_context(tc.tile_pool(name="fpsum", bufs=2, space="PSUM"))
            e_pool = es.enter_context(tc.tile_pool(name="expS", bufs=14))
            r_pool = es.enter_context(tc.tile_pool(name="recip", bufs=2))
            st_pool = es.enter_context(tc.tile_pool(name="stage", bufs=3))
            rb_pool = es.enter_context(tc.tile_pool(name="recipb", bufs=2))
            out_pool = es.enter_context(tc.tile_pool(name="outc", bufs=3))

            # ---- persistent SBUF tiles ----------------------------------
            xT = [persist.tile([128, N], bf16, name=f"xT{k}", tag=f"xT{k}")
                  for k in range(KT)]
            wqk0a = persist.tile([128, 256], bf16, name="wqk0a", tag="wqk0a")
            wqk0b = persist.tile([128, 1280], bf16, name="wqk0b", tag="wqk0b")
            wqkA = persist.tile([128, KT * 512], bf16, name="wqkA", tag="wqkA")
            wqkB = persist.tile([128, KT * 768], bf16, name="wqkB", tag="wqkB")
            wv = persist.tile([128, KT * C], bf16, name="wv", tag="wv")
            wp = persist.tile([128, KT * C], bf16, name="wp", tag="wp")
            ball = persist.tile([128, 18], f32, name="ball", tag="ball")
            bv = persist.tile([128, H, D], f32, name="bv", tag="bv")
            ones64 = persist.tile([1, 64], f32, name="ones64", tag="ones64")
            qkT = [persist.tile([128, N], bf16, name=f"qkT{m}", tag=f"qkT{m}")
                   for m in range(12)]
            v_sb = [persist.tile([128, H, D + 1], bf16, name=f"v{t}", tag=f"v{t}")
                    for t in range(MT)]
            oT = [persist.tile([128, N], bf16, name=f"oT{m}", tag=f"oT{m}")
                  for m in range(KT)]

            # ---- input DMAs, spread across sequencers -------------------
            # (HWDGE only exists on SP + Activation; gpsimd uses SWDGE.)
            # The 16 DMA queues are FIFO and shared: a big transfer issued
            # early blocks later-issued critical ones, so everything large
            # queues BEHIND the xT chunks the prelude needs first.
            nc.scalar.dma_start(wqk0a[:], wqk0a_e[:])
            nc.scalar.dma_start(wqk0b[:], wqk0b_e[:])
            for k in range(KT):
                nc.sync.dma_start(xT[k][:], xT_e[:, k, :])
            nc.sync.dma_start(wqkA[:], wqkA_e[:])
            nc.sync.dma_start(wqkB[:], wqkB_e[:])
            nc.sync.dma_start(wp[:], wp_e[:])
            # gpsimd: biases + V weights
            nc.gpsimd.dma_start(ball[:], ball_e[:])
            nc.gpsimd.dma_start(wv[:], wv_e[:])
            nc.gpsimd.dma_start(bv[:], bv_e[:])
            nc.gpsimd.memset(ones64[:], 1.0)

            # lhsT slice of [W_q*SCALE | W_k] for contraction chunk k, tile m
            def qk_lhsT(k, m):
                p = m % 6
                kcol = 128 if m >= 6 else 0
                if p == 0:
                    if k == 0:
                        return wqk0a[:, kcol:kcol + 128]
                    return wqk0b[:, 256 * (k - 1) + kcol:256 * (k - 1) + kcol + 128]
                if p <= 2:
                    o = 512 * k + 256 * (p - 1) + kcol
                    return wqkA[:, o:o + 128]
                o = 768 * k + 256 * (p - 3) + kcol
                return wqkB[:, o:o + 128]

            # ---- filler sub-chunks (1-bank PSUM each, ~6 MMs) -----------
            def sub_qk(m, j):
                def emit():
                    ps = f_pool.tile([128, 512], f32, name="fps", tag="fps")
                    for k in range(KT):
                        nc.tensor.matmul(
                            ps[:],
                            lhsT=qk_lhsT(k, m),
                            rhs=xT[k][:, 512 * j:512 * (j + 1)],
                            start=(k == 0), stop=(k == KT - 1),
                        )
                    nc.vector.tensor_scalar_add(
                        qkT[m][:, 512 * j:512 * (j + 1)], ps[:], ball[:, m:m + 1])
                return emit

            def sub_v(t, part):
                c0, cw = ((0, 512), (512, 256))[part]
                h0, hn = ((0, 8), (8, 4))[part]
                def emit():
                    ps = f_pool.tile([128, 512], f32, name="fps", tag="fps")
                    for k in range(KT):
                        nc.tensor.matmul(
                            ps[:, 0:cw],
                            lhsT=xT[k][:, 128 * t:128 * (t + 1)],
                            rhs=wv[:, C * k + c0:C * k + c0 + cw],
                            start=(k == 0), stop=(k == KT - 1),
                        )
                    if part == 0:
                        nc.gpsimd.memset(v_sb[t][:, :, D:D + 1], 1.0)
                    nc.vector.tensor_add(
                        v_sb[t][:, h0:h0 + hn, 0:D],
                        ps[:, 0:cw].rearrange("p (h x) -> p h x", x=D),
                        bv[:, h0:h0 + hn, :],
                    )
                return emit

            ph3_out = [[persist.tile([128, 512], f32, name=f"p3_{c}_{j}",
                                     tag=f"p3_{c}_{j}") for j in range(NJ)]
                       for c in range(KT)]

            def wp_lhsT(m, c):
                return wp[:, C * m + 128 * c:C * m + 128 * c + 128]

            def sub_ph3a(c, j):
                def emit():
                    ps = f_pool.tile([128, 512], f32, name="fps", tag="fps")
                    for m in range(4):
                        nc.tensor.matmul(
                            ps[:],
                            lhsT=wp_lhsT(m, c),
                            rhs=oT[m][:, 512 * j:512 * (j + 1)],
                            start=(m == 0), stop=(m == 3),
                        )
                    nc.vector.tensor_scalar_add(
                        ph3_out[c][j][:], ps[:], ball[:, 12 + c:13 + c])
                return emit

            # ---- head-pair machinery ------------------------------------
            def s_step(pair, m, e_e, e_o):
                """4 S matmuls alternating row-halves + 2 exps."""
                qt, kt = qkT[pair], qkT[6 + pair]
                t_e = s_pool.tile([128, N], f32, name="sps", tag="sps")
                t_o = s_pool.tile([128, N], f32, name="sps", tag="sps")
                for j in range(NJ):
                    for po, t in ((0, t_e), (64, t_o)):
                        nc.tensor.matmul(
                            t[:, 512 * j:512 * (j + 1)],
                            lhsT=kt[po:po + 64, 128 * m:128 * (m + 1)],
                            rhs=qt[po:po + 64, 512 * j:512 * (j + 1)],
                            start=True, stop=True,
                        )
                for t, lst in ((t_e, e_e), (t_o, e_o)):
                    e_sb = e_pool.tile([128, N], bf16, name="e_sb", tag="e_sb")
                    nc.scalar.activation(e_sb[:], t[:], Exp)
                    lst.append(e_sb)

            class HeadPV:
                """Trailing PV + normalize for one head, consumed task-wise."""
                def __init__(self, h, e_tiles):
                    self.h, self.e = h, e_tiles
                    self.m = 0
                    self.o_ps = [o_pool.tile([65, 512], f32, name="o_ps",
                                             tag="o_ps") for _ in range(NJ)]

                def step(self):
                    h, m = self.h, self.m
                    for j in range(NJ):
                        nc.tensor.matmul(
                            self.o_ps[j][:, :],
                            lhsT=v_sb[m][:, h, :],
                            rhs=self.e[m][:, 512 * j:512 * (j + 1)],
                            start=(m == 0), stop=(m == MT - 1),
                        )
                    self.m += 1
                    if self.m == MT:
                        self.finish()
                        return True
                    return False

                def finish(self):
                    # stage PSUM->SBUF first so the o accumulator banks free
                    # earlier; the normalize chain then runs off SBUF.
                    h, po = self.h, 64 * (self.h % 2)
                    sc = r_pool.tile([1, N], f32, name="sc", tag="sc")
                    st = st_pool.tile([64, N], f32, name="st", tag="st")
                    if h == 11:
                        # ScalarE is idle once its exp stream ends: stage the
                        # last pair there, and run the whole normalize per-j
                        # so the j=0 slice of oT[5] lands earlier (the
                        # proj tail consumes j-outer).  The partition
                        # broadcast runs as a tiny PE matmul (ones^T @ r,
                        # 216ns) instead of GpSimd (~1us) — latency is what
                        # matters on this chain.
                        for j in range(NJ):
                            jsl = slice(512 * j, 512 * (j + 1))
                            nc.scalar.activation(
                                sc[0:1, jsl], self.o_ps[j][64:65, :], CopyF)
                            nc.scalar.activation(
                                st[0:64, jsl], self.o_ps[j][0:64, :], CopyF)
                            r = r_pool.tile([1, 512], f32, name="r", tag="r")
                            nc.vector.reciprocal_approx_fast(
                                r[0:1, :], sc[0:1, jsl])
                            rb = f_pool.tile([64, 512], f32, name="rbp",
                                             tag="fps")
                            nc.tensor.matmul(
                                rb[:, :], lhsT=ones64[0:1, :], rhs=r[0:1, :],
                                start=True, stop=True)
                            nc.vector.tensor_mul(
                                oT[h // 2][po:po + 64, jsl],
                                st[0:64, jsl], rb[0:64, :])
                        return
                    for j in range(NJ):
                        nc.vector.tensor_copy(
                            sc[0:1, 512 * j:512 * (j + 1)], self.o_ps[j][64:65, :])
                        nc.vector.tensor_copy(
                            st[0:64, 512 * j:512 * (j + 1)], self.o_ps[j][0:64, :])
                    r = r_pool.tile([1, N], f32, name="r", tag="r")
                    nc.vector.reciprocal_approx_fast(r[0:1, :], sc[0:1, :])
                    rb = rb_pool.tile([64, N], f32, name="rb", tag="rb")
                    nc.gpsimd.partition_broadcast(rb[:], r[0:1, :])
                    for j in range(NJ):
                        nc.vector.tensor_mul(
                            oT[h // 2][po:po + 64, 512 * j:512 * (j + 1)],
                            st[0:64, 512 * j:512 * (j + 1)],
                            rb[0:64, 512 * j:512 * (j + 1)],
                        )

            # ---- the software-pipelined schedule ------------------------
            from collections import deque
            fillers = deque()
            pv_queue = deque()   # HeadPV objects, strictly ordered

            def drain_pv(max_tasks):
                n = 0
                while pv_queue and n < max_tasks:
                    hp = pv_queue[0]
                    if hp.m >= len(hp.e):
                        break  # exp for this m not emitted yet
                    if hp.step():
                        pv_queue.popleft()
                    n += 1

            def drain_fillers(max_chunks):
                for _ in range(min(max_chunks, len(fillers))):
                    fillers.popleft()()

            # proj partial chunks (m=0..3), emitted directly in order once
            # heads 0-7 have retired.  Never queued into `fillers`.
            proj_pend = deque(
                [sub_ph3a(c, j) for c in range(KT) for j in range(NJ)])

            def proj_gate_ok():
                return bool(proj_pend) and                        not any(hp.h <= 7 for hp in pv_queue)

            # prelude: QK chunks for pair 0 (PE warmup, un-gated)
            for j in range(NJ):
                sub_qk(0, j)()
            for j in range(NJ):
                sub_qk(6, j)()
            # fillers for pair 0: all V sub-chunks, then pair-1 QK chunks
            for t in range(MT):
                fillers.append(sub_v(t, 0))
                fillers.append(sub_v(t, 1))
            for m in (1, 7):
                for j in range(NJ):
                    fillers.append(sub_qk(m, j))

            for pair in range(5):
                e_e, e_o = [], []
                pend_e, pend_o = HeadPV(2 * pair, e_e), HeadPV(2 * pair + 1, e_o)
                if pair < 4:
                    new_fill = [sub_qk(pair + 2, j) for j in range(NJ)] + \
                               [sub_qk(6 + pair + 2, j) for j in range(NJ)]
                else:
                    new_fill = []
                for m in range(MT):
                    s_step(pair, m, e_e, e_o)
                    if m == 2:
                        pv_queue.append(pend_e)
                        pv_queue.append(pend_o)
                    drain_pv(2)
                    nfill = 3 if pair == 0 else 1
                    drain_fillers(nfill)
                    if new_fill and m % 4 == 1:
                        fillers.append(new_fill.pop(0))
                        fillers.append(new_fill.pop(0))
                    # once heads 0-7 retire, feed proj partials into the
                    # filler-less pair 4 (only when qk/v fillers are done)
                    if pair == 4 and not fillers and proj_gate_ok():
                        proj_pend.popleft()()

            # pair 5 runs head-serial (head 10 fully, then head 11) so the
            # last head's PV + normalize trail its own exp stream by ~1
            # block instead of queueing behind head 10's PSUM accumulators.
            qt5, kt5 = qkT[5], qkT[11]
            for half in range(2):
                po = 64 * half
                e_lst = []
                pend = HeadPV(10 + half, e_lst)
                for m in range(MT):
                    t = s_pool.tile([128, N], f32, name="sps", tag="sps")
                    for j in range(NJ):
                        nc.tensor.matmul(
                            t[:, 512 * j:512 * (j + 1)],
                            lhsT=kt5[po:po + 64, 128 * m:128 * (m + 1)],
                            rhs=qt5[po:po + 64, 512 * j:512 * (j + 1)],
                            start=True, stop=True,
                        )
                    e_sb = e_pool.tile([128, N], bf16, name="e_sb", tag="e_sb")
                    nc.scalar.activation(e_sb[:], t[:], Exp)
                    e_lst.append(e_sb)
                    if m == 1:
                        pv_queue.append(pend)
                    drain_pv(2)
                    drain_fillers(1)
                    if proj_gate_ok():
                        proj_pend.popleft()()
            # drain what remains, interleaving the remaining proj chunks
            while pv_queue:
                drain_pv(4)
                if proj_gate_ok():
                    proj_pend.popleft()()
                drain_fillers(1)
            drain_fillers(len(fillers))
            while proj_pend:
                proj_pend.popleft()()

            # ---- phase 3 tail: add the m=5 contribution + DMA out -------
            dma_engines = [nc.gpsimd, nc.sync, nc.scalar]
            for j in range(NJ):
                for c in range(KT):
                    ps = f_pool.tile([128, 512], f32, name="fps", tag="fps")
                    for mi, m in enumerate((4, 5)):
                        nc.tensor.matmul(
                            ps[:],
                            lhsT=wp_lhsT(m, c),
                            rhs=oT[m][:, 512 * j:512 * (j + 1)],
                            start=(mi == 0), stop=(mi == 1),
                        )
                    oc = out_pool.tile([128, 512], bf16, name="oc", tag="oc")
                    # (GpSimd cannot read PSUM — birverifier rejects it —
                    # so all 12 final adds stay on DVE)
                    nc.vector.tensor_add(oc[:], ps[:], ph3_out[c][j][:])
                    eng = dma_engines[(j * KT + c) % 3]
                    eng.dma_start(
                        out_e[128 * c:128 * (c + 1), 512 * j:512 * (j + 1)], oc[:])

    nc.compile()
    return nc


def prep_inputs(x, W_qkv, b_qkv, W_proj, b_proj):
    """Host-side shard + layout prep. Returns in_maps for 8 cores."""
    x = np.asarray(x, dtype=np.float32)
    W_qkv = np.asarray(W_qkv, dtype=np.float32)
    b_qkv = np.asarray(b_qkv, dtype=np.float32)
    W_proj = np.asarray(W_proj, dtype=np.float32)
    b_proj = np.asarray(b_proj, dtype=np.float32)

    w_qk = np.concatenate([W_qkv[:, :C] * SCALE, W_qkv[:, C:2 * C]], axis=1)
    w_qk = np.ascontiguousarray(w_qk).astype(BF16)      # [768, 1536]

    def kmajor(w):  # [768, F] -> [128, KT*F]
        f = w.shape[1]
        return np.ascontiguousarray(
            w.reshape(KT, 128, f).transpose(1, 0, 2).reshape(128, KT * f))

    # pair-major column grouping: pair p holds [Q_p (128) | K_p (128)]
    pairs = [np.concatenate([w_qk[:, 128 * p:128 * (p + 1)],
                             w_qk[:, 768 + 128 * p:768 + 128 * (p + 1)]],
                            axis=1) for p in range(6)]       # each [768, 256]
    wqk0 = kmajor(pairs[0])                                  # [128, 1536]
    wqk0a = np.ascontiguousarray(wqk0[:, :256])              # k=0
    wqk0b = np.ascontiguousarray(wqk0[:, 256:])              # k=1..5
    wqkA = kmajor(np.concatenate(pairs[1:3], axis=1))        # [128, 3072]
    wqkB = kmajor(np.concatenate(pairs[3:6], axis=1))        # [128, 4608]

    w_v = kmajor(np.ascontiguousarray(W_qkv[:, 2 * C:]).astype(BF16))
    w_p = kmajor(W_proj.astype(BF16))

    b_qk = np.concatenate([b_qkv[:C] * SCALE, b_qkv[C:2 * C]])
    b_all = np.empty((128, 18), np.float32)
    b_all[:, :12] = b_qk.reshape(12, 128).T
    b_all[:, 12:] = b_proj.reshape(6, 128).T
    b_v = np.ascontiguousarray(
        np.broadcast_to(b_qkv[2 * C:].reshape(H, D), (128, H, D))).astype(np.float32)

    shared = {"wqk0a": wqk0a, "wqk0b": wqk0b, "wqkA": wqkA, "wqkB": wqkB,
              "w_v": w_v, "w_proj": w_p, "b_all": b_all, "b_v": b_v}
    in_maps = []
    for b in range(NCORES):
        xT = np.ascontiguousarray(x[b].T).astype(BF16)       # [768, 1024]
        m = dict(shared)
        m["xT"] = np.ascontiguousarray(
            xT.reshape(KT, 128, N).transpose(1, 0, 2))       # [128, 6, 1024]
        in_maps.append(m)
    return in_maps


def kernel(x, W_qkv, b_qkv, W_proj, b_proj):
    from concourse.bass_utils import run_bass_kernel_spmd

    nc = _CACHE.get("nc")
    if nc is None:
        nc = _CACHE["nc"] = build_nc()

    in_maps = prep_inputs(x, W_qkv, b_qkv, W_proj, b_proj)
    res = run_bass_kernel_spmd(nc, in_maps, core_ids=list(range(NCORES)))
    out = np.empty((B, N, C), np.float32)
    for b in range(NCORES):
        out[b] = res.results[b]["outT"].astype(np.float32).T
    return out


# revision 16
# speedup vs baseline: 1.0585x; 1.0137x over previous
"""Trainium2 Bass kernel for batch-8 multi-head attention.

Strategy: pure data parallelism — one batch element per NeuronCore (B=8,
8 cores), zero collectives.  All inputs are pre-arranged on the host so the
device kernel only ever runs dense matmuls in its preferred layouts.

  per-core DRAM inputs (bf16 unless noted):
    xT        [128, 6, 1024]  x[b].T k-chunked       (feature-major activations)
    wqk0a     [128, 256]      pair-0 (m=0,6) cols of [W_q*SCALE | W_k], k=0
    wqk0b     [128, 1280]     pair-0 cols, k=1..5
    wqkA      [128, 3072]     pairs 1-2 cols, k-major
    wqkB      [128, 4608]     pairs 3-5 cols, k-major
    wv        [128, 4608]     W_v k-major
    wp        [128, 4608]     W_proj k-major
    b_all     [128, 18] f32   per-partition bias chunks (12 qk + 6 proj)
    b_v       [128, 12, 64] f32  V bias broadcast along partitions
  output:
    outT      [768, 1024] bf16  (attention output)^T — host transposes back

Device pipeline per core (program order interleaves phases so ScalarE's exp
stream starts early and the PE never starves — PE occupancy is the
bottleneck: ~165us of matmul on a ~143.5us column-count floor):
  QK^T = w_qk^T @ xT                 -> 12 tiles [128, 1024], 2 heads/tile
  V    = xT^T @ w_v + b_v            -> 8 tiles [128, 12, 65], ones col fused
  per head h:
    S^T[m]   = K_h @ Q_h^T           (K=64 contraction)
    expS     = exp(S^T)  on ScalarE  (no max subtraction: |logits| < ~8)
    [O^T|s]  = [V_m|1]^T @ expS      (PSUM accumulate over m; row 64 = sums)
    oT       = O^T * (1/s)           (recip_approx + partition-broadcast + mul)
  outT = w_proj^T @ oT + b_proj      (partials m0-3 early, m4 after heads 8/9,
                                      m5-only tail after the last normalize)

DMA plan: input dma_starts are spread across the scalar/sync/vector/gpsimd
sequencers (one DIRECT2D is ~610ns of sequencer time — serializing 26 of
them on SP delayed PE saturation to ~25us).  Output chunks round-robin over
all four sequencers as they complete.

PSUM: s-tiles 2-bank x2 + o-accumulators 1-bank x2 + filler chunks 1-bank x2
= 8 banks, shared across all phases with no pool barrier.

Attempts that CORRUPT on hardware (sim-clean, do not retry): genuinely
overlapping 64-row-tiled S matmuls with neighboring 128-mode matmuls
(tile_position + early PSUM release), [65,512] DVE copies from PSUM, and
reciprocal/mul reading o_ps PSUM directly.  Finer [128,512] exps are
correct but slower (+287ns fixed cost per activation instruction).
"""

import os
import sys

os.environ.setdefault("BASS_PERFETTO_PROFILE_ALL_CORES", "1")
if "/opt/trn_rl_repo" not in sys.path:
    sys.path.insert(0, "/opt/trn_rl_repo")

import numpy as np
import ml_dtypes

B, N, C, H = 8, 1024, 768, 12
D = C // H                # 64 head dim
SCALE = D ** -0.5
NCORES = 8
KT = C // 128             # 6 contraction tiles over C
MT = N // 128             # 8 token blocks
NJ = N // 512             # 2 query chunks of 512
BF16 = ml_dtypes.bfloat16

_CACHE = {}


def build_nc():
    """Build + compile the per-core Bass graph (identical on all 8 cores)."""
    import concourse.tile as tile
    from concourse import bacc, mybir

    f32 = mybir.dt.float32
    bf16 = mybir.dt.bfloat16
    Exp = mybir.ActivationFunctionType.Exp
    CopyF = mybir.ActivationFunctionType.Copy

    nc = bacc.Bacc("TRN2", target_bir_lowering=False, debug=False,
                   num_devices=NCORES)

    xT_e = nc.dram_tensor("xT", [128, KT, N], bf16, kind="ExternalInput").ap()
    wqk0a_e = nc.dram_tensor("wqk0a", [128, 256], bf16, kind="ExternalInput").ap()
    wqk0b_e = nc.dram_tensor("wqk0b", [128, 1280], bf16, kind="ExternalInput").ap()
    wqkA_e = nc.dram_tensor("wqkA", [128, KT * 512], bf16, kind="ExternalInput").ap()
    wqkB_e = nc.dram_tensor("wqkB", [128, KT * 768], bf16, kind="ExternalInput").ap()
    wv_e = nc.dram_tensor("w_v", [128, KT * C], bf16, kind="ExternalInput").ap()
    wp_e = nc.dram_tensor("w_proj", [128, KT * C], bf16, kind="ExternalInput").ap()
    ball_e = nc.dram_tensor("b_all", [128, 18], f32, kind="ExternalInput").ap()
    bv_e = nc.dram_tensor("b_v", [128, H, D], f32, kind="ExternalInput").ap()
    out_e = nc.dram_tensor("outT", [C, N], bf16, kind="ExternalOutput").ap()

    with tile.TileContext(nc) as tc:
        from contextlib import ExitStack

        with ExitStack() as es:
            persist = es.enter_context(tc.tile_pool(name="persist", bufs=1))
            s_pool = es.enter_context(tc.tile_pool(name="spsum", bufs=2, space="PSUM"))
            o_pool = es.enter_context(tc.tile_pool(name="opsum", bufs=2, space="PSUM"))
            f_pool = es.enter_context(tc.tile_pool(name="fpsum", bufs=2, space="PSUM"))
            e_pool = es.enter_context(tc.tile_pool(name="expS", bufs=14))
            r_pool = es.enter_context(tc.tile_pool(name="recip", bufs=2))
            st_pool = es.enter_context(tc.tile_pool(name="stage", bufs=3))
            rb_pool = es.enter_context(tc.tile_pool(name="recipb", bufs=2))
            out_pool = es.enter_context(tc.tile_pool(name="outc", bufs=3))

            # ---- persistent SBUF tiles ----------------------------------
            xT = [persist.tile([128, N], bf16, name=f"xT{k}", tag=f"xT{k}")
                  for k in range(KT)]
            wqk0a = persist.tile([128, 256], bf16, name="wqk0a", tag="wqk0a")
            wqk0b = persist.tile([128, 1280], bf16, name="wqk0b", tag="wqk0b")
            wqkA = persist.tile([128, KT * 512], bf16, name="wqkA", tag="wqkA")
            wqkB = persist.tile([128, KT * 768], bf16, name="wqkB", tag="wqkB")
            wv = persist.tile([128, KT * C], bf16, name="wv", tag="wv")
            wp = persist.tile([128, KT * C], bf16, name="wp", tag="wp")
            ball = persist.tile([128, 18], f32, name="ball", tag="ball")
            bv = persist.tile([128, H, D], f32, name="bv", tag="bv")
            qkT = [persist.tile([128, N], bf16, name=f"qkT{m}", tag=f"qkT{m}")
                   for m in range(12)]
            v_sb = [persist.tile([128, H, D + 1], bf16, name=f"v{t}", tag=f"v{t}")
                    for t in range(MT)]
            oT = [persist.tile([128, N], bf16, name=f"oT{m}", tag=f"oT{m}")
                  for m in range(KT)]

            # ---- input DMAs, spread across sequencers -------------------
            # (HWDGE only exists on SP + Activation; gpsimd uses SWDGE.)
            # The 16 DMA queues are FIFO and shared: a big transfer issued
            # early blocks later-issued critical ones, so everything large
            # queues BEHIND the xT chunks the prelude needs first.
            nc.scalar.dma_start(wqk0a[:], wqk0a_e[:])
            nc.scalar.dma_start(wqk0b[:], wqk0b_e[:])
            for k in range(KT):
                nc.sync.dma_start(xT[k][:], xT_e[:, k, :])
            nc.sync.dma_start(wqkA[:], wqkA_e[:])
            nc.sync.dma_start(wqkB[:], wqkB_e[:])
            nc.sync.dma_start(wp[:], wp_e[:])
            # gpsimd: biases + V weights
            nc.gpsimd.dma_start(ball[:], ball_e[:])
            nc.gpsimd.dma_start(wv[:], wv_e[:])
            nc.gpsimd.dma_start(bv[:], bv_e[:])

            # lhsT slice of [W_q*SCALE | W_k] for contraction chunk k, tile m
            def qk_lhsT(k, m):
                p = m % 6
                kcol = 128 if m >= 6 else 0
                if p == 0:
                    if k == 0:
                        return wqk0a[:, kcol:kcol + 128]
                    return wqk0b[:, 256 * (k - 1) + kcol:256 * (k - 1) + kcol + 128]
                if p <= 2:
                    o = 512 * k + 256 * (p - 1) + kcol
                    return wqkA[:, o:o + 128]
                o = 768 * k + 256 * (p - 3) + kcol
                return wqkB[:, o:o + 128]

            # ---- filler sub-chunks (1-bank PSUM each, ~6 MMs) -----------
            def sub_qk(m, j):
                def emit():
                    ps = f_pool.tile([128, 512], f32, name="fps", tag="fps")
                    for k in range(KT):
                        nc.tensor.matmul(
                            ps[:],
                            lhsT=qk_lhsT(k, m),
                            rhs=xT[k][:, 512 * j:512 * (j + 1)],
                            start=(k == 0), stop=(k == KT - 1),
                        )
                    nc.vector.tensor_scalar_add(
                        qkT[m][:, 512 * j:512 * (j + 1)], ps[:], ball[:, m:m + 1])
                return emit

            def sub_v(t, part):
                c0, cw = ((0, 512), (512, 256))[part]
                h0, hn = ((0, 8), (8, 4))[part]
                def emit():
                    ps = f_pool.tile([128, 512], f32, name="fps", tag="fps")
                    for k in range(KT):
                        nc.tensor.matmul(
                            ps[:, 0:cw],
                            lhsT=xT[k][:, 128 * t:128 * (t + 1)],
                            rhs=wv[:, C * k + c0:C * k + c0 + cw],
                            start=(k == 0), stop=(k == KT - 1),
                        )
                    if part == 0:
                        nc.gpsimd.memset(v_sb[t][:, :, D:D + 1], 1.0)
                    nc.vector.tensor_add(
                        v_sb[t][:, h0:h0 + hn, 0:D],
                        ps[:, 0:cw].rearrange("p (h x) -> p h x", x=D),
                        bv[:, h0:h0 + hn, :],
                    )
                return emit

            ph3_out = [[persist.tile([128, 512], f32, name=f"p3_{c}_{j}",
                                     tag=f"p3_{c}_{j}") for j in range(NJ)]
                       for c in range(KT)]

            def wp_lhsT(m, c):
                return wp[:, C * m + 128 * c:C * m + 128 * c + 128]

            def sub_ph3a(c, j):
                def emit():
                    ps = f_pool.tile([128, 512], f32, name="fps", tag="fps")
                    for m in range(4):
                        nc.tensor.matmul(
                            ps[:],
                            lhsT=wp_lhsT(m, c),
                            rhs=oT[m][:, 512 * j:512 * (j + 1)],
                            start=(m == 0), stop=(m == 3),
                        )
                    nc.vector.tensor_scalar_add(
                        ph3_out[c][j][:], ps[:], ball[:, 12 + c:13 + c])
                return emit

            # ---- head-pair machinery ------------------------------------
            def s_step(pair, m, e_e, e_o):
                """4 S matmuls alternating row-halves + 2 exps."""
                qt, kt = qkT[pair], qkT[6 + pair]
                t_e = s_pool.tile([128, N], f32, name="sps", tag="sps")
                t_o = s_pool.tile([128, N], f32, name="sps", tag="sps")
                for j in range(NJ):
                    for po, t in ((0, t_e), (64, t_o)):
                        nc.tensor.matmul(
                            t[:, 512 * j:512 * (j + 1)],
                            lhsT=kt[po:po + 64, 128 * m:128 * (m + 1)],
                            rhs=qt[po:po + 64, 512 * j:512 * (j + 1)],
                            start=True, stop=True,
                        )
                for t, lst in ((t_e, e_e), (t_o, e_o)):
                    e_sb = e_pool.tile([128, N], bf16, name="e_sb", tag="e_sb")
                    nc.scalar.activation(e_sb[:], t[:], Exp)
                    lst.append(e_sb)

            class HeadPV:
                """Trailing PV + normalize for one head, consumed task-wise."""
                def __init__(self, h, e_tiles):
                    self.h, self.e = h, e_tiles
                    self.m = 0
                    self.o_ps = [o_pool.tile([65, 512], f32, name="o_ps",
                                             tag="o_ps") for _ in range(NJ)]

                def step(self):
                    h, m = self.h, self.m
                    for j in range(NJ):
                        nc.tensor.matmul(
                            self.o_ps[j][:, :],
                            lhsT=v_sb[m][:, h, :],
                            rhs=self.e[m][:, 512 * j:512 * (j + 1)],
                            start=(m == 0), stop=(m == MT - 1),
                        )
                    self.m += 1
                    if self.m == MT:
                        self.finish()
                        return True
                    return False

                def finish(self):
                    # stage PSUM->SBUF first so the o accumulator banks free
                    # earlier; the normalize chain then runs off SBUF.
                    h, po = self.h, 64 * (self.h % 2)
                    sc = r_pool.tile([1, N], f32, name="sc", tag="sc")
                    st = st_pool.tile([64, N], f32, name="st", tag="st")
                    if h == 11:
                        # ScalarE is idle once its exp stream ends: stage the
                        # last pair there, and run the whole normalize per-j
                        # so the j=0 slice of oT[5] lands earlier (the
                        # proj tail consumes j-outer).
                        for j in range(NJ):
                            jsl = slice(512 * j, 512 * (j + 1))
                            nc.scalar.activation(
                                sc[0:1, jsl], self.o_ps[j][64:65, :], CopyF)
                            nc.scalar.activation(
                                st[0:64, jsl], self.o_ps[j][0:64, :], CopyF)
                            r = r_pool.tile([1, 512], f32, name="r", tag="r")
                            nc.vector.reciprocal_approx_fast(
                                r[0:1, :], sc[0:1, jsl])
                            rb = rb_pool.tile([64, 512], f32, name="rb", tag="rb")
                            nc.gpsimd.partition_broadcast(rb[:], r[0:1, :])
                            nc.vector.tensor_mul(
                                oT[h // 2][po:po + 64, jsl],
                                st[0:64, jsl], rb[0:64, :])
                        return
                    for j in range(NJ):
                        nc.vector.tensor_copy(
                            sc[0:1, 512 * j:512 * (j + 1)], self.o_ps[j][64:65, :])
                        nc.vector.tensor_copy(
                            st[0:64, 512 * j:512 * (j + 1)], self.o_ps[j][0:64, :])
                    r = r_pool.tile([1, N], f32, name="r", tag="r")
                    nc.vector.reciprocal_approx_fast(r[0:1, :], sc[0:1, :])
                    rb = rb_pool.tile([64, N], f32, name="rb", tag="rb")
                    nc.gpsimd.partition_broadcast(rb[:], r[0:1, :])
                    for j in range(NJ):
                        nc.vector.tensor_mul(
                            oT[h // 2][po:po + 64, 512 * j:512 * (j + 1)],
                            st[0:64, 512 * j:512 * (j + 1)],
                            rb[0:64, 512 * j:512 * (j + 1)],
                        )

            # ---- the software-pipelined schedule ------------------------
            from collections import deque
            fillers = deque()
            pv_queue = deque()   # HeadPV objects, strictly ordered

            def drain_pv(max_tasks):
                n = 0
                while pv_queue and n < max_tasks:
                    hp = pv_queue[0]
                    if hp.m >= len(hp.e):
                        break  # exp for this m not emitted yet
                    if hp.step():
                        pv_queue.popleft()
                    n += 1

            def drain_fillers(max_chunks):
                for _ in range(min(max_chunks, len(fillers))):
                    fillers.popleft()()

            # proj partial chunks (m=0..3), emitted directly in order once
            # heads 0-7 have retired.  Never queued into `fillers`.
            proj_pend = deque(
                [sub_ph3a(c, j) for c in range(KT) for j in range(NJ)])

            def proj_gate_ok():
                return bool(proj_pend) and                        not any(hp.h <= 7 for hp in pv_queue)

            # prelude: QK chunks for pair 0 (PE warmup, un-gated)
            for j in range(NJ):
                sub_qk(0, j)()
            for j in range(NJ):
                sub_qk(6, j)()
            # fillers for pair 0: all V sub-chunks, then pair-1 QK chunks
            for t in range(MT):
                fillers.append(sub_v(t, 0))
                fillers.append(sub_v(t, 1))
            for m in (1, 7):
                for j in range(NJ):
                    fillers.append(sub_qk(m, j))

            for pair in range(5):
                e_e, e_o = [], []
                pend_e, pend_o = HeadPV(2 * pair, e_e), HeadPV(2 * pair + 1, e_o)
                if pair < 4:
                    new_fill = [sub_qk(pair + 2, j) for j in range(NJ)] + \
                               [sub_qk(6 + pair + 2, j) for j in range(NJ)]
                else:
                    new_fill = []
                for m in range(MT):
                    s_step(pair, m, e_e, e_o)
                    if m == 2:
                        pv_queue.append(pend_e)
                        pv_queue.append(pend_o)
                    drain_pv(2)
                    nfill = 3 if pair == 0 else 1
                    drain_fillers(nfill)
                    if new_fill and m % 4 == 1:
                        fillers.append(new_fill.pop(0))
                        fillers.append(new_fill.pop(0))
                    # once heads 0-7 retire, feed proj partials into the
                    # filler-less pair 4 (only when qk/v fillers are done)
                    if pair == 4 and not fillers and proj_gate_ok():
                        proj_pend.popleft()()

            # pair 5 runs head-serial (head 10 fully, then head 11) so the
            # last head's PV + normalize trail its own exp stream by ~1
            # block instead of queueing behind head 10's PSUM accumulators.
            qt5, kt5 = qkT[5], qkT[11]
            for half in range(2):
                po = 64 * half
                e_lst = []
                pend = HeadPV(10 + half, e_lst)
                for m in range(MT):
                    t = s_pool.tile([128, N], f32, name="sps", tag="sps")
                    for j in range(NJ):
                        nc.tensor.matmul(
                            t[:, 512 * j:512 * (j + 1)],
                            lhsT=kt5[po:po + 64, 128 * m:128 * (m + 1)],
                            rhs=qt5[po:po + 64, 512 * j:512 * (j + 1)],
                            start=True, stop=True,
                        )
                    e_sb = e_pool.tile([128, N], bf16, name="e_sb", tag="e_sb")
                    nc.scalar.activation(e_sb[:], t[:], Exp)
                    e_lst.append(e_sb)
                    if m == 1:
                        pv_queue.append(pend)
                    drain_pv(2)
                    drain_fillers(1)
                    # feed sparsely: extra PE work here delays the last exp;
                    # leftovers run in the drain under the Vector tail
                    if m % 3 == 2 and proj_gate_ok():
                        proj_pend.popleft()()
            # drain: finish the last PV + normalize first (its Vector ops
            # must precede the proj/tail adds in the DVE queue), then the
            # leftover proj chunks (they keep the PE warm under the
            # normalize), then the tail.
            while pv_queue:
                drain_pv(4)
            drain_fillers(len(fillers))
            while proj_pend:
                proj_pend.popleft()()

            # ---- phase 3 tail: add the m=5 contribution + DMA out -------
            dma_engines = [nc.gpsimd, nc.sync, nc.scalar]
            for j in range(NJ):
                for c in range(KT):
                    ps = f_pool.tile([128, 512], f32, name="fps", tag="fps")
                    for mi, m in enumerate((4, 5)):
                        nc.tensor.matmul(
                            ps[:],
                            lhsT=wp_lhsT(m, c),
                            rhs=oT[m][:, 512 * j:512 * (j + 1)],
                            start=(mi == 0), stop=(mi == 1),
                        )
                    oc = out_pool.tile([128, 512], bf16, name="oc", tag="oc")
                    # (GpSimd cannot read PSUM — birverifier rejects it —
                    # so all 12 final adds stay on DVE)
                    nc.vector.tensor_add(oc[:], ps[:], ph3_out[c][j][:])
                    eng = dma_engines[(j * KT + c) % 3]
                    eng.dma_start(
                        out_e[128 * c:128 * (c + 1), 512 * j:512 * (j + 1)], oc[:])

    nc.compile()
    return nc


def prep_inputs(x, W_qkv, b_qkv, W_proj, b_proj):
    """Host-side shard + layout prep. Returns in_maps for 8 cores."""
    x = np.asarray(x, dtype=np.float32)
    W_qkv = np.asarray(W_qkv, dtype=np.float32)
    b_qkv = np.asarray(b_qkv, dtype=np.float32)
    W_proj = np.asarray(W_proj, dtype=np.float32)
    b_proj = np.asarray(b_proj, dtype=np.float32)

    w_qk = np.concatenate([W_qkv[:, :C] * SCALE, W_qkv[:, C:2 * C]], axis=1)
    w_qk = np.ascontiguousarray(w_qk).astype(BF16)      # [768, 1536]

    def kmajor(w):  # [768, F] -> [128, KT*F]
        f = w.shape[1]
        return np.ascontiguousarray(
            w.reshape(KT, 128, f).transpose(1, 0, 2).reshape(128, KT * f))

    # pair-major column grouping: pair p holds [Q_p (128) | K_p (128)]
    pairs = [np.concatenate([w_qk[:, 128 * p:128 * (p + 1)],
                             w_qk[:, 768 + 128 * p:768 + 128 * (p + 1)]],
                            axis=1) for p in range(6)]       # each [768, 256]
    wqk0 = kmajor(pairs[0])                                  # [128, 1536]
    wqk0a = np.ascontiguousarray(wqk0[:, :256])              # k=0
    wqk0b = np.ascontiguousarray(wqk0[:, 256:])              # k=1..5
    wqkA = kmajor(np.concatenate(pairs[1:3], axis=1))        # [128, 3072]
    wqkB = kmajor(np.concatenate(pairs[3:6], axis=1))        # [128, 4608]

    w_v = kmajor(np.ascontiguousarray(W_qkv[:, 2 * C:]).astype(BF16))
    w_p = kmajor(W_proj.astype(BF16))

    b_qk = np.concatenate([b_qkv[:C] * SCALE, b_qkv[C:2 * C]])
    b_all = np.empty((128, 18), np.float32)
    b_all[:, :12] = b_qk.reshape(12, 128).T
    b_all[:, 12:] = b_proj.reshape(6, 128).T
    b_v = np.ascontiguousarray(
        np.broadcast_to(b_qkv[2 * C:].reshape(H, D), (128, H, D))).astype(np.float32)

    shared = {"wqk0a": wqk0a, "wqk0b": wqk0b, "wqkA": wqkA, "wqkB": wqkB,
              "w_v": w_v, "w_proj": w_p, "b_all": b_all, "b_v": b_v}
    in_maps = []
    for b in range(NCORES):
        xT = np.ascontiguousarray(x[b].T).astype(BF16)       # [768, 1024]
        m = dict(shared)
        m["xT"] = np.ascontiguousarray(
            xT.reshape(KT, 128, N).transpose(1, 0, 2))       # [128, 6, 1024]
        in_maps.append(m)
    return in_maps


def kernel(x, W_qkv, b_qkv, W_proj, b_proj):
    from concourse.bass_utils import run_bass_kernel_spmd

    nc = _CACHE.get("nc")
    if nc is None:
        nc = _CACHE["nc"] = build_nc()

    in_maps = prep_inputs(x, W_qkv, b_qkv, W_proj, b_proj)
    res = run_bass_kernel_spmd(nc, in_maps, core_ids=list(range(NCORES)))
    out = np.empty((B, N, C), np.float32)
    for b in range(NCORES):
        out[b] = res.results[b]["outT"].astype(np.float32).T
    return out


# revision 17
# speedup vs baseline: 1.0643x; 1.0054x over previous
"""Trainium2 Bass kernel for batch-8 multi-head attention.

Strategy: pure data parallelism — one batch element per NeuronCore (B=8,
8 cores), zero collectives.  All inputs are pre-arranged on the host so the
device kernel only ever runs dense matmuls in its preferred layouts.

  per-core DRAM inputs (bf16 unless noted):
    xT        [128, 6, 1024]  x[b].T k-chunked       (feature-major activations)
    wqk0a     [128, 256]      pair-0 (m=0,6) cols of [W_q*SCALE | W_k], k=0
    wqk0b     [128, 1280]     pair-0 cols, k=1..5
    wqkA      [128, 3072]     pairs 1-2 cols, k-major
    wqkB      [128, 4608]     pairs 3-5 cols, k-major
    wv        [128, 4608]     W_v k-major
    wp        [128, 4608]     W_proj k-major
    b_all     [128, 18] f32   per-partition bias chunks (12 qk + 6 proj)
    b_v       [128, 12, 64] f32  V bias broadcast along partitions
  output:
    outT      [768, 1024] bf16  (attention output)^T — host transposes back

Device pipeline per core (program order interleaves phases so ScalarE's exp
stream starts early and the PE never starves — PE occupancy is the
bottleneck: ~165us of matmul on a ~143.5us column-count floor):
  QK^T = w_qk^T @ xT                 -> 12 tiles [128, 1024], 2 heads/tile
  V    = xT^T @ w_v + b_v            -> 8 tiles [128, 12, 65], ones col fused
  per head h:
    S^T[m]   = K_h @ Q_h^T           (K=64 contraction)
    expS     = exp(S^T)  on ScalarE  (no max subtraction: |logits| < ~8)
    [O^T|s]  = [V_m|1]^T @ expS      (PSUM accumulate over m; row 64 = sums)
    oT       = O^T * (1/s)           (recip_approx + partition-broadcast + mul)
  outT = w_proj^T @ oT + b_proj      (partials m0-3 early, m4 after heads 8/9,
                                      m5-only tail after the last normalize)

DMA plan: input dma_starts are spread across the scalar/sync/vector/gpsimd
sequencers (one DIRECT2D is ~610ns of sequencer time — serializing 26 of
them on SP delayed PE saturation to ~25us).  Output chunks round-robin over
all four sequencers as they complete.

PSUM: s-tiles 2-bank x2 + o-accumulators 1-bank x2 + filler chunks 1-bank x2
= 8 banks, shared across all phases with no pool barrier.

Attempts that CORRUPT on hardware (sim-clean, do not retry): genuinely
overlapping 64-row-tiled S matmuls with neighboring 128-mode matmuls
(tile_position + early PSUM release), [65,512] DVE copies from PSUM, and
reciprocal/mul reading o_ps PSUM directly.  Finer [128,512] exps are
correct but slower (+287ns fixed cost per activation instruction).
"""

import os
import sys

os.environ.setdefault("BASS_PERFETTO_PROFILE_ALL_CORES", "1")
if "/opt/trn_rl_repo" not in sys.path:
    sys.path.insert(0, "/opt/trn_rl_repo")

import numpy as np
import ml_dtypes

B, N, C, H = 8, 1024, 768, 12
D = C // H                # 64 head dim
SCALE = D ** -0.5
NCORES = 8
KT = C // 128             # 6 contraction tiles over C
MT = N // 128             # 8 token blocks
NJ = N // 512             # 2 query chunks of 512
BF16 = ml_dtypes.bfloat16

_CACHE = {}


def build_nc():
    """Build + compile the per-core Bass graph (identical on all 8 cores)."""
    import concourse.tile as tile
    from concourse import bacc, mybir

    f32 = mybir.dt.float32
    bf16 = mybir.dt.bfloat16
    Exp = mybir.ActivationFunctionType.Exp
    CopyF = mybir.ActivationFunctionType.Copy

    nc = bacc.Bacc("TRN2", target_bir_lowering=False, debug=False,
                   num_devices=NCORES)

    xT_e = nc.dram_tensor("xT", [128, KT, N], bf16, kind="ExternalInput").ap()
    wqk0a_e = nc.dram_tensor("wqk0a", [128, 256], bf16, kind="ExternalInput").ap()
    wqk0b_e = nc.dram_tensor("wqk0b", [128, 1280], bf16, kind="ExternalInput").ap()
    wqkA_e = nc.dram_tensor("wqkA", [128, KT * 512], bf16, kind="ExternalInput").ap()
    wqkB_e = nc.dram_tensor("wqkB", [128, KT * 768], bf16, kind="ExternalInput").ap()
    wv_e = nc.dram_tensor("w_v", [128, KT * C], bf16, kind="ExternalInput").ap()
    wp_e = nc.dram_tensor("w_proj", [128, KT * C], bf16, kind="ExternalInput").ap()
    ball_e = nc.dram_tensor("b_all", [128, 18], f32, kind="ExternalInput").ap()
    bv_e = nc.dram_tensor("b_v", [128, H, D], f32, kind="ExternalInput").ap()
    out_e = nc.dram_tensor("outT", [C, N], bf16, kind="ExternalOutput").ap()

    with tile.TileContext(nc) as tc:
        from contextlib import ExitStack

        with ExitStack() as es:
            persist = es.enter_context(tc.tile_pool(name="persist", bufs=1))
            s_pool = es.enter_context(tc.tile_pool(name="spsum", bufs=2, space="PSUM"))
            o_pool = es.enter_context(tc.tile_pool(name="opsum", bufs=2, space="PSUM"))
            f_pool = es.enter_context(tc.tile_pool(name="fpsum", bufs=2, space="PSUM"))
            e_pool = es.enter_context(tc.tile_pool(name="expS", bufs=14))
            r_pool = es.enter_context(tc.tile_pool(name="recip", bufs=2))
            st_pool = es.enter_context(tc.tile_pool(name="stage", bufs=3))
            rb_pool = es.enter_context(tc.tile_pool(name="recipb", bufs=2))
            out_pool = es.enter_context(tc.tile_pool(name="outc", bufs=3))

            # ---- persistent SBUF tiles ----------------------------------
            xT = [persist.tile([128, N], bf16, name=f"xT{k}", tag=f"xT{k}")
                  for k in range(KT)]
            wqk0a = persist.tile([128, 256], bf16, name="wqk0a", tag="wqk0a")
            wqk0b = persist.tile([128, 1280], bf16, name="wqk0b", tag="wqk0b")
            wqkA = persist.tile([128, KT * 512], bf16, name="wqkA", tag="wqkA")
            wqkB = persist.tile([128, KT * 768], bf16, name="wqkB", tag="wqkB")
            wv = persist.tile([128, KT * C], bf16, name="wv", tag="wv")
            wp = persist.tile([128, KT * C], bf16, name="wp", tag="wp")
            ball = persist.tile([128, 18], f32, name="ball", tag="ball")
            bv = persist.tile([128, H, D], f32, name="bv", tag="bv")
            qkT = [persist.tile([128, N], bf16, name=f"qkT{m}", tag=f"qkT{m}")
                   for m in range(12)]
            v_sb = [persist.tile([128, H, D + 1], bf16, name=f"v{t}", tag=f"v{t}")
                    for t in range(MT)]
            oT = [persist.tile([128, N], bf16, name=f"oT{m}", tag=f"oT{m}")
                  for m in range(KT)]

            # ---- input DMAs, spread across sequencers -------------------
            # (HWDGE only exists on SP + Activation; gpsimd uses SWDGE.)
            # The 16 DMA queues are FIFO and shared: a big transfer issued
            # early blocks later-issued critical ones, so everything large
            # queues BEHIND the xT chunks the prelude needs first.
            nc.scalar.dma_start(wqk0a[:], wqk0a_e[:])
            nc.scalar.dma_start(wqk0b[:], wqk0b_e[:])
            for k in range(KT):
                nc.sync.dma_start(xT[k][:], xT_e[:, k, :])
            nc.sync.dma_start(wv[:], wv_e[:])
            nc.sync.dma_start(wqkA[:], wqkA_e[:])
            nc.sync.dma_start(wqkB[:], wqkB_e[:])
            nc.sync.dma_start(wp[:], wp_e[:])
            # gpsimd: biases (wv would clog the DMA queues ahead of xT)
            nc.gpsimd.dma_start(ball[:], ball_e[:])
            nc.gpsimd.dma_start(bv[:], bv_e[:])

            # PE p-state warmup: ~3.3us of throwaway matmuls during the
            # initial DMA wait so the first real matmuls run at 2.4GHz
            # (any idle gap drops the PE to 1.2GHz for the next ~3us).
            junk = persist.tile([128, 256], bf16, name="junk", tag="junk")
            nc.gpsimd.memset(junk[:], 0.0)
            for _ in range(14):
                jps = f_pool.tile([128, 512], f32, name="fps", tag="fps")
                nc.tensor.matmul(jps[:, 0:256], lhsT=junk[:, 0:128],
                                 rhs=junk[:], start=True, stop=True)

            # lhsT slice of [W_q*SCALE | W_k] for contraction chunk k, tile m
            def qk_lhsT(k, m):
                p = m % 6
                kcol = 128 if m >= 6 else 0
                if p == 0:
                    if k == 0:
                        return wqk0a[:, kcol:kcol + 128]
                    return wqk0b[:, 256 * (k - 1) + kcol:256 * (k - 1) + kcol + 128]
                if p <= 2:
                    o = 512 * k + 256 * (p - 1) + kcol
                    return wqkA[:, o:o + 128]
                o = 768 * k + 256 * (p - 3) + kcol
                return wqkB[:, o:o + 128]

            # ---- filler sub-chunks (1-bank PSUM each, ~6 MMs) -----------
            def sub_qk(m, j):
                def emit():
                    ps = f_pool.tile([128, 512], f32, name="fps", tag="fps")
                    for k in range(KT):
                        nc.tensor.matmul(
                            ps[:],
                            lhsT=qk_lhsT(k, m),
                            rhs=xT[k][:, 512 * j:512 * (j + 1)],
                            start=(k == 0), stop=(k == KT - 1),
                        )
                    nc.vector.tensor_scalar_add(
                        qkT[m][:, 512 * j:512 * (j + 1)], ps[:], ball[:, m:m + 1])
                return emit

            def sub_v(t, part):
                c0, cw = ((0, 512), (512, 256))[part]
                h0, hn = ((0, 8), (8, 4))[part]
                def emit():
                    ps = f_pool.tile([128, 512], f32, name="fps", tag="fps")
                    for k in range(KT):
                        nc.tensor.matmul(
                            ps[:, 0:cw],
                            lhsT=xT[k][:, 128 * t:128 * (t + 1)],
                            rhs=wv[:, C * k + c0:C * k + c0 + cw],
                            start=(k == 0), stop=(k == KT - 1),
                        )
                    if part == 0:
                        nc.gpsimd.memset(v_sb[t][:, :, D:D + 1], 1.0)
                    nc.vector.tensor_add(
                        v_sb[t][:, h0:h0 + hn, 0:D],
                        ps[:, 0:cw].rearrange("p (h x) -> p h x", x=D),
                        bv[:, h0:h0 + hn, :],
                    )
                return emit

            ph3_out = [[persist.tile([128, 512], f32, name=f"p3_{c}_{j}",
                                     tag=f"p3_{c}_{j}") for j in range(NJ)]
                       for c in range(KT)]

            def wp_lhsT(m, c):
                return wp[:, C * m + 128 * c:C * m + 128 * c + 128]

            def sub_ph3a(c, j):
                def emit():
                    ps = f_pool.tile([128, 512], f32, name="fps", tag="fps")
                    for m in range(4):
                        nc.tensor.matmul(
                            ps[:],
                            lhsT=wp_lhsT(m, c),
                            rhs=oT[m][:, 512 * j:512 * (j + 1)],
                            start=(m == 0), stop=(m == 3),
                        )
                    nc.vector.tensor_scalar_add(
                        ph3_out[c][j][:], ps[:], ball[:, 12 + c:13 + c])
                return emit

            # ---- head-pair machinery ------------------------------------
            def s_step(pair, m, e_e, e_o):
                """4 S matmuls alternating row-halves + 2 exps."""
                qt, kt = qkT[pair], qkT[6 + pair]
                t_e = s_pool.tile([128, N], f32, name="sps", tag="sps")
                t_o = s_pool.tile([128, N], f32, name="sps", tag="sps")
                # half-major: both j chunks of a half share one stationary
                # (and one PE tile position), halving LDWEIGHTS exposure
                for po, t in ((0, t_e), (64, t_o)):
                    for j in range(NJ):
                        nc.tensor.matmul(
                            t[:, 512 * j:512 * (j + 1)],
                            lhsT=kt[po:po + 64, 128 * m:128 * (m + 1)],
                            rhs=qt[po:po + 64, 512 * j:512 * (j + 1)],
                            start=True, stop=True,
                        )
                for t, lst in ((t_e, e_e), (t_o, e_o)):
                    e_sb = e_pool.tile([128, N], bf16, name="e_sb", tag="e_sb")
                    nc.scalar.activation(e_sb[:], t[:], Exp)
                    lst.append(e_sb)

            class HeadPV:
                """Trailing PV + normalize for one head, consumed task-wise."""
                def __init__(self, h, e_tiles):
                    self.h, self.e = h, e_tiles
                    self.m = 0
                    self.o_ps = [o_pool.tile([65, 512], f32, name="o_ps",
                                             tag="o_ps") for _ in range(NJ)]

                def step(self):
                    h, m = self.h, self.m
                    for j in range(NJ):
                        nc.tensor.matmul(
                            self.o_ps[j][:, :],
                            lhsT=v_sb[m][:, h, :],
                            rhs=self.e[m][:, 512 * j:512 * (j + 1)],
                            start=(m == 0), stop=(m == MT - 1),
                        )
                    self.m += 1
                    if self.m == MT:
                        self.finish()
                        return True
                    return False

                def finish(self):
                    # stage PSUM->SBUF first so the o accumulator banks free
                    # earlier; the normalize chain then runs off SBUF.
                    h, po = self.h, 64 * (self.h % 2)
                    sc = r_pool.tile([1, N], f32, name="sc", tag="sc")
                    st = st_pool.tile([64, N], f32, name="st", tag="st")
                    if h == 11:
                        # ScalarE is idle once its exp stream ends: stage the
                        # last pair there, and run the whole normalize per-j
                        # so the j=0 slice of oT[5] lands earlier (the
                        # proj tail consumes j-outer).
                        for j in range(NJ):
                            jsl = slice(512 * j, 512 * (j + 1))
                            nc.scalar.activation(
                                sc[0:1, jsl], self.o_ps[j][64:65, :], CopyF)
                            nc.scalar.activation(
                                st[0:64, jsl], self.o_ps[j][0:64, :], CopyF)
                            r = r_pool.tile([1, 512], f32, name="r", tag="r")
                            nc.vector.reciprocal_approx_fast(
                                r[0:1, :], sc[0:1, jsl])
                            rb = rb_pool.tile([64, 512], f32, name="rb", tag="rb")
                            nc.gpsimd.partition_broadcast(rb[:], r[0:1, :])
                            nc.vector.tensor_mul(
                                oT[h // 2][po:po + 64, jsl],
                                st[0:64, jsl], rb[0:64, :])
                        return
                    for j in range(NJ):
                        nc.vector.tensor_copy(
                            sc[0:1, 512 * j:512 * (j + 1)], self.o_ps[j][64:65, :])
                        nc.vector.tensor_copy(
                            st[0:64, 512 * j:512 * (j + 1)], self.o_ps[j][0:64, :])
                    r = r_pool.tile([1, N], f32, name="r", tag="r")
                    nc.vector.reciprocal_approx_fast(r[0:1, :], sc[0:1, :])
                    rb = rb_pool.tile([64, N], f32, name="rb", tag="rb")
                    nc.gpsimd.partition_broadcast(rb[:], r[0:1, :])
                    for j in range(NJ):
                        nc.vector.tensor_mul(
                            oT[h // 2][po:po + 64, 512 * j:512 * (j + 1)],
                            st[0:64, 512 * j:512 * (j + 1)],
                            rb[0:64, 512 * j:512 * (j + 1)],
                        )

            # ---- the software-pipelined schedule ------------------------
            from collections import deque
            fillers = deque()
            pv_queue = deque()   # HeadPV objects, strictly ordered

            def drain_pv(max_tasks):
                n = 0
                while pv_queue and n < max_tasks:
                    hp = pv_queue[0]
                    if hp.m >= len(hp.e):
                        break  # exp for this m not emitted yet
                    if hp.step():
                        pv_queue.popleft()
                    n += 1

            def drain_fillers(max_chunks):
                for _ in range(min(max_chunks, len(fillers))):
                    fillers.popleft()()

            # proj partial chunks (m=0..3), emitted directly in order once
            # heads 0-7 have retired.  Never queued into `fillers`.
            proj_pend = deque(
                [sub_ph3a(c, j) for c in range(KT) for j in range(NJ)])

            def proj_gate_ok():
                return bool(proj_pend) and                        not any(hp.h <= 7 for hp in pv_queue)

            # prelude: QK chunks for pair 0 (PE warmup, un-gated)
            for j in range(NJ):
                sub_qk(0, j)()
            for j in range(NJ):
                sub_qk(6, j)()
            # fillers for pair 0: all V sub-chunks, then pair-1 QK chunks
            for t in range(MT):
                fillers.append(sub_v(t, 0))
                fillers.append(sub_v(t, 1))
            for m in (1, 7):
                for j in range(NJ):
                    fillers.append(sub_qk(m, j))

            for pair in range(5):
                e_e, e_o = [], []
                pend_e, pend_o = HeadPV(2 * pair, e_e), HeadPV(2 * pair + 1, e_o)
                if pair < 4:
                    new_fill = [sub_qk(pair + 2, j) for j in range(NJ)] + \
                               [sub_qk(6 + pair + 2, j) for j in range(NJ)]
                else:
                    new_fill = []
                for m in range(MT):
                    s_step(pair, m, e_e, e_o)
                    if m == 2:
                        pv_queue.append(pend_e)
                        pv_queue.append(pend_o)
                    drain_pv(2)
                    nfill = 3 if pair == 0 else 1
                    drain_fillers(nfill)
                    if new_fill and m % 4 == 1:
                        fillers.append(new_fill.pop(0))
                        fillers.append(new_fill.pop(0))
                    # once heads 0-7 retire, feed proj partials into the
                    # filler-less pair 4 (only when qk/v fillers are done)
                    if pair == 4 and not fillers and proj_gate_ok():
                        proj_pend.popleft()()

            # pair 5 runs head-serial (head 10 fully, then head 11) so the
            # last head's PV + normalize trail its own exp stream by ~1
            # block instead of queueing behind head 10's PSUM accumulators.
            qt5, kt5 = qkT[5], qkT[11]
            for half in range(2):
                po = 64 * half
                e_lst = []
                pend = HeadPV(10 + half, e_lst)
                for m in range(MT):
                    t = s_pool.tile([128, N], f32, name="sps", tag="sps")
                    for j in range(NJ):
                        nc.tensor.matmul(
                            t[:, 512 * j:512 * (j + 1)],
                            lhsT=kt5[po:po + 64, 128 * m:128 * (m + 1)],
                            rhs=qt5[po:po + 64, 512 * j:512 * (j + 1)],
                            start=True, stop=True,
                        )
                    e_sb = e_pool.tile([128, N], bf16, name="e_sb", tag="e_sb")
                    nc.scalar.activation(e_sb[:], t[:], Exp)
                    e_lst.append(e_sb)
                    if m == 1:
                        pv_queue.append(pend)
                    drain_pv(2)
                    drain_fillers(1)
                    # feed sparsely: extra PE work here delays the last exp;
                    # leftovers run in the drain under the Vector tail
                    if m % 4 == 3 and proj_gate_ok():
                        proj_pend.popleft()()
            # drain: finish the last PV + normalize first (its Vector ops
            # must precede the proj/tail adds in the DVE queue), then the
            # leftover proj chunks (they keep the PE warm under the
            # normalize), then the tail.
            while pv_queue:
                drain_pv(4)
            drain_fillers(len(fillers))
            while proj_pend:
                proj_pend.popleft()()

            # ---- phase 3 tail: add the m=5 contribution + DMA out -------
            dma_engines = [nc.gpsimd, nc.sync, nc.scalar]
            for j in range(NJ):
                for c in range(KT):
                    ps = f_pool.tile([128, 512], f32, name="fps", tag="fps")
                    for mi, m in enumerate((4, 5)):
                        nc.tensor.matmul(
                            ps[:],
                            lhsT=wp_lhsT(m, c),
                            rhs=oT[m][:, 512 * j:512 * (j + 1)],
                            start=(mi == 0), stop=(mi == 1),
                        )
                    oc = out_pool.tile([128, 512], bf16, name="oc", tag="oc")
                    # (GpSimd cannot read PSUM — birverifier rejects it —
                    # so all 12 final adds stay on DVE)
                    nc.vector.tensor_add(oc[:], ps[:], ph3_out[c][j][:])
                    eng = dma_engines[(j * KT + c) % 3]
                    eng.dma_start(
                        out_e[128 * c:128 * (c + 1), 512 * j:512 * (j + 1)], oc[:])

    nc.compile()
    return nc


def prep_inputs(x, W_qkv, b_qkv, W_proj, b_proj):
    """Host-side shard + layout prep. Returns in_maps for 8 cores."""
    x = np.asarray(x, dtype=np.float32)
    W_qkv = np.asarray(W_qkv, dtype=np.float32)
    b_qkv = np.asarray(b_qkv, dtype=np.float32)
    W_proj = np.asarray(W_proj, dtype=np.float32)
    b_proj = np.asarray(b_proj, dtype=np.float32)

    w_qk = np.concatenate([W_qkv[:, :C] * SCALE, W_qkv[:, C:2 * C]], axis=1)
    w_qk = np.ascontiguousarray(w_qk).astype(BF16)      # [768, 1536]

    def kmajor(w):  # [768, F] -> [128, KT*F]
        f = w.shape[1]
        return np.ascontiguousarray(
            w.reshape(KT, 128, f).transpose(1, 0, 2).reshape(128, KT * f))

    # pair-major column grouping: pair p holds [Q_p (128) | K_p (128)]
    pairs = [np.concatenate([w_qk[:, 128 * p:128 * (p + 1)],
                             w_qk[:, 768 + 128 * p:768 + 128 * (p + 1)]],
                            axis=1) for p in range(6)]       # each [768, 256]
    wqk0 = kmajor(pairs[0])                                  # [128, 1536]
    wqk0a = np.ascontiguousarray(wqk0[:, :256])              # k=0
    wqk0b = np.ascontiguousarray(wqk0[:, 256:])              # k=1..5
    wqkA = kmajor(np.concatenate(pairs[1:3], axis=1))        # [128, 3072]
    wqkB = kmajor(np.concatenate(pairs[3:6], axis=1))        # [128, 4608]

    w_v = kmajor(np.ascontiguousarray(W_qkv[:, 2 * C:]).astype(BF16))
    w_p = kmajor(W_proj.astype(BF16))

    b_qk = np.concatenate([b_qkv[:C] * SCALE, b_qkv[C:2 * C]])
    b_all = np.empty((128, 18), np.float32)
    b_all[:, :12] = b_qk.reshape(12, 128).T
    b_all[:, 12:] = b_proj.reshape(6, 128).T
    b_v = np.ascontiguousarray(
        np.broadcast_to(b_qkv[2 * C:].reshape(H, D), (128, H, D))).astype(np.float32)

    shared = {"wqk0a": wqk0a, "wqk0b": wqk0b, "wqkA": wqkA, "wqkB": wqkB,
              "w_v": w_v, "w_proj": w_p, "b_all": b_all, "b_v": b_v}
    in_maps = []
    for b in range(NCORES):
        xT = np.ascontiguousarray(x[b].T).astype(BF16)       # [768, 1024]
        m = dict(shared)
        m["xT"] = np.ascontiguousarray(
            xT.reshape(KT, 128, N).transpose(1, 0, 2))       # [128, 6, 1024]
        in_maps.append(m)
    return in_maps


def kernel(x, W_qkv, b_qkv, W_proj, b_proj):
    from concourse.bass_utils import run_bass_kernel_spmd

    nc = _CACHE.get("nc")
    if nc is None:
        nc = _CACHE["nc"] = build_nc()

    in_maps = prep_inputs(x, W_qkv, b_qkv, W_proj, b_proj)
    res = run_bass_kernel_spmd(nc, in_maps, core_ids=list(range(NCORES)))
    out = np.empty((B, N, C), np.float32)
    for b in range(NCORES):
        out[b] = res.results[b]["outT"].astype(np.float32).T
    return out
